# revision 21
# baseline (speedup 1.0000x reference)
"""Trainium2 Bass kernel for CausalSelfAttention (sliding window 256 + 4 sink).

v4: 8 cores = (batch 2) x (seq chunks of 512). Per core:
KV = 772 = [own 512 | halo 256 | sink 4], QKV + RoPE in bf16, banded
attention in scores-transposed [k, q] layout with 7 key chunks.

Changes over v3:
- PE warmup matmuls right after the first x tile lands (HAM K=8/8 by the
  time real work starts).
- AV col-split: both head-halves' AV chains run concurrently in the PE
  array (half0 -> psum partitions 0:64, half1 -> 64:128 via tile_position),
  yt is ONE [128, 512] psum tile, evacuated with ONE Act copy.
- Denominator+broadcast fused: ones[m, 64] stationary -> the PE both
  row-sums pt AND replicates the result across 64 partitions; the two
  halves' chains run concurrently. No dh copies, no sel broadcast MMs.
- reciprocal_approx_fast (1 DVE op, fp32) instead of iterative RECIPROCAL
  (3.4us -> ~0.7us, off the critical path).
- masks multiplied per group (in-place into praw) so AV starts earlier.
- V stored plain [128, 1024] (no ones column, no strided copies).
- proj psum evacuation on DVE (Act is the busier engine there).
"""

import numpy as np

B, T, C, NH, HD = 2, 2048, 1024, 16, 64
WIN, SINK = 256, 4
CH = 512          # queries per core
KV = 772          # 512 own + 256 halo + 4 sink
KVP = 784         # KV padded so fp8 pair-stride is 16B-aligned (DoubleRow ISA)
NCORES = 8
W_C = [384, 384, 256, 128, 128, 256, 512]
OFF_C = [0, 128, 256, 384, 0, 0, 0]
# Score chunks packed into psum banks of 512 cols each; praw/mask share
# this column layout. Sink group first: chunk 6 is the AV starter.
GROUPS = [[6], [0, 3], [1, 4], [2, 5]]
MOFF = np.zeros(8, int)
_off = 0
for _g in GROUPS:
    for _c in _g:
        MOFF[_c] = _off
        _off += W_C[_c]
MTOT = _off  # 2048
AVORD = [6, 0, 3, 1, 4, 2, 5]
CROWS = [128, 128, 128, 128, 128, 128, 4]
CBASE = [0, 128, 256, 384, 512, 640, 768]
KSEG = [(0, 512), (512, 260)]  # kv free-dim segments (psum bank limit)
NWARM = 14

_cache = {}


def _build_nc():
    import concourse.bacc as bacc
    import concourse.mybir as mybir
    import concourse.tile as tile

    f32 = mybir.dt.float32
    bf16 = mybir.dt.bfloat16
    fp8 = mybir.dt.float8e4
    DR = mybir.MatmulPerfMode.DoubleRow
    AF = mybir.ActivationFunctionType

    nc = bacc.Bacc("TRN2", target_bir_lowering=False, debug=False,
                   num_devices=NCORES)

    xT = nc.dram_tensor("xT", [C, KV], bf16, kind="ExternalInput").ap()
    wqs = nc.dram_tensor("wqs", [C, C], bf16, kind="ExternalInput").ap()
    wks = nc.dram_tensor("wks", [C, C], bf16, kind="ExternalInput").ap()
    wv = nc.dram_tensor("wv", [C, C], bf16, kind="ExternalInput").ap()
    wps = nc.dram_tensor("wps", [C, C], bf16, kind="ExternalInput").ap()
    cos_q = nc.dram_tensor("cos_q", [128, CH], bf16, kind="ExternalInput").ap()
    sin_q = nc.dram_tensor("sin_q", [128, CH], bf16, kind="ExternalInput").ap()
    cos_k = nc.dram_tensor("cos_k", [128, KV], bf16, kind="ExternalInput").ap()
    sin_k = nc.dram_tensor("sin_k", [128, KV], bf16, kind="ExternalInput").ap()
    masks = nc.dram_tensor("masks", [128, MTOT], bf16,
                           kind="ExternalInput").ap()
    p2d = nc.dram_tensor("p2", [128, 128], bf16, kind="ExternalInput").ap()
    outT = nc.dram_tensor("outT", [C, CH], f32, kind="ExternalOutput").ap()

    with tile.TileContext(nc) as tc:
        with (
            tc.tile_pool(name="pers", bufs=1) as pers,
            tc.tile_pool(name="qk", bufs=3) as qkp,
            tc.tile_pool(name="tmp", bufs=2) as tmp,
            tc.tile_pool(name="praw", bufs=2) as prp,
            tc.tile_pool(name="rcp", bufs=2) as rcpp,
            tc.tile_pool(name="osb", bufs=2) as osbp,
            tc.tile_pool(name="psmm", bufs=2, space="PSUM") as psmm,
            tc.tile_pool(name="pssc", bufs=3, space="PSUM") as pssc,
            tc.tile_pool(name="psyd", bufs=3, space="PSUM") as psyd,
        ):
            # ones block: denominator chains use cols 0:64; also the PE
            # warmup operand (ready ~immediately, unlike any DMA)
            tones = pers.tile([128, 512], bf16, tag="ones")
            nc.gpsimd.memset(tones[:], 1.0)

            # PE warmup: HAM un-throttles after ~3.4us of sustained PE
            # activity; burn the DMA-load window on dummy matmuls so real
            # matmuls run at 2.4 GHz from the start.
            for wi in range(NWARM):
                pw = psmm.tile([128, 512], f32, tag="mm", name=f"warm{wi}")
                nc.tensor.matmul(pw[:], tones[:, 0:128], tones[:],
                                 start=True, stop=True)

            # x split: query columns first (gates the first Q projection),
            # halo+sink columns after the first weight tiles.
            xq, xh = [], []
            for i in range(8):
                t = pers.tile([128, CH], bf16, tag=f"xq{i}", name=f"xq{i}")
                nc.sync.dma_start(t[:], xT[i * 128:(i + 1) * 128, 0:CH])
                xq.append(t)

            wqsl, wksl = [None] * 8, [None] * 8

            def load_wqk(i):
                t = pers.tile([128, 1024], bf16, tag=f"wq{i}", name=f"wq{i}")
                nc.sync.dma_start(t[:], wqs[i * 128:(i + 1) * 128, :])
                wqsl[i] = t
                t = pers.tile([128, 1024], bf16, tag=f"wk{i}", name=f"wk{i}")
                nc.sync.dma_start(t[:], wks[i * 128:(i + 1) * 128, :])
                wksl[i] = t

            load_wqk(0)
            tcos_q = pers.tile([128, CH], bf16, tag="cos_q")
            nc.sync.dma_start(tcos_q[:], cos_q[:])
            tsin_q = pers.tile([128, CH], bf16, tag="sin_q")
            nc.sync.dma_start(tsin_q[:], sin_q[:])
            tp2 = pers.tile([128, 128], bf16, tag="p2")
            nc.sync.dma_start(tp2[:], p2d[:])
            for i in range(8):
                t = pers.tile([128, KV - CH], bf16, tag=f"xh{i}",
                              name=f"xh{i}")
                nc.sync.dma_start(t[:], xT[i * 128:(i + 1) * 128, CH:KV])
                xh.append(t)
            tcos_k = pers.tile([128, KV], bf16, tag="cos_k")
            nc.sync.dma_start(tcos_k[:], cos_k[:])
            tsin_k = pers.tile([128, KV], bf16, tag="sin_k")
            nc.sync.dma_start(tsin_k[:], sin_k[:])
            wvc = []
            for i in range(8):
                t = pers.tile([128, 1024], bf16, tag=f"wvc{i}", name=f"wvc{i}")
                nc.sync.dma_start(t[:], wv[i * 128:(i + 1) * 128, :])
                wvc.append(t)
            load_wqk(1)
            tmask = pers.tile([128, MTOT], bf16, tag="mask")
            nc.sync.dma_start(tmask[:], masks[:])
            for i in range(2, 8):
                load_wqk(i)
            wpsl = []
            for i in range(8):
                t = pers.tile([128, 1024], bf16, tag=f"wp{i}", name=f"wp{i}")
                nc.sync.dma_start(t[:], wps[i * 128:(i + 1) * 128, :])
                wpsl.append(t)

            def qkv_rope(hp):
                # qT raw -> rope
                pq = psmm.tile([128, 512], f32, tag="mm", name=f"pq{hp}")
                for kc in range(8):
                    nc.tensor.matmul(
                        pq[:], wqsl[hp][:, kc * 128:(kc + 1) * 128],
                        xq[kc][:],
                        start=(kc == 0), stop=(kc == 7),
                    )
                qraw = tmp.tile([128, CH], bf16, tag="qraw", name=f"qraw{hp}")
                nc.scalar.copy(qraw[:], pq[:])
                qT = qkp.tile([128, CH], bf16, tag="qT", name=f"qT{hp}")
                prot = psmm.tile([128, 512], f32, tag="mm", name=f"prot{hp}")
                nc.tensor.matmul(prot[:], tp2[:], qraw[:], start=True,
                                 stop=True)
                t2 = tmp.tile([128, CH], bf16, tag="t2", name=f"t2q{hp}")
                nc.vector.tensor_mul(t2[:], prot[:], tsin_q[:])
                qm = tmp.tile([128, CH], bf16, tag="qm", name=f"qm{hp}")
                nc.vector.tensor_mul(qm[:], qraw[:], tcos_q[:])
                nc.vector.tensor_add(qT[:], qm[:], t2[:])

                # kT raw -> rope (two psum segments)
                kraw = tmp.tile([128, KV], bf16, tag="kraw", name=f"kraw{hp}")
                pk = [psmm.tile([128, 512], f32, tag="mm", name=f"pk{hp}_{i}")
                      for i in range(2)]
                for kc in range(8):
                    for si, (s0, sw) in enumerate(KSEG):
                        rhs = xq[kc][:] if si == 0 else xh[kc][:, 0:sw]
                        nc.tensor.matmul(
                            pk[si][:, 0:sw],
                            wksl[hp][:, kc * 128:(kc + 1) * 128],
                            rhs,
                            start=(kc == 0), stop=(kc == 7),
                        )
                kT = qkp.tile([128, KV], bf16, tag="kT", name=f"kT{hp}")
                for si, (s0, sw) in enumerate(KSEG):
                    nc.scalar.copy(kraw[:, s0:s0 + sw], pk[si][:, 0:sw])
                    prk = psmm.tile([128, 512], f32, tag="mm",
                                    name=f"prk{hp}_{si}")
                    nc.tensor.matmul(prk[:, 0:sw], tp2[:],
                                     kraw[:, s0:s0 + sw], start=True,
                                     stop=True)
                    t2k = tmp.tile([128, 512], bf16, tag="t2",
                                   name=f"t2k{hp}_{si}")
                    nc.vector.tensor_mul(t2k[:, 0:sw], prk[:, 0:sw],
                                         tsin_k[:, s0:s0 + sw])
                    km = tmp.tile([128, 512], bf16, tag="qm",
                                  name=f"km{hp}_{si}")
                    nc.vector.tensor_mul(km[:, 0:sw], kraw[:, s0:s0 + sw],
                                         tcos_k[:, s0:s0 + sw])
                    nc.vector.tensor_add(kT[:, s0:s0 + sw], km[:, 0:sw],
                                         t2k[:, 0:sw])
                return qT, kT

            def sc_exp_block(hp, qT, kT):
                # scoresT: per chunk, two head-halves at partition bases 0/64
                # run concurrently in the PE array. Chunks within a GROUP
                # share one 512-wide psum bank, so each (group, half) needs
                # only ONE exp (fused 1/8 scale) into the contiguous
                # [128, MTOT] praw tile per half.
                praws = []
                for half in range(2):
                    praws.append(prp.tile([128, MTOT], bf16, tag="praw",
                                          name=f"praw{hp}_{half}"))
                for gi, grp in enumerate(GROUPS):
                    g0 = MOFF[grp[0]]
                    gw = sum(W_C[c] for c in grp)
                    gm = max(CROWS[c] for c in grp)
                    for half in range(2):
                        dsl = slice(half * 64, half * 64 + 64)
                        sc = pssc.tile([128, 512], f32, tag="sc",
                                       name=f"sc{hp}_{gi}_{half}")
                        for c in grp:
                            w, off, m = W_C[c], OFF_C[c], CROWS[c]
                            base = CBASE[c]
                            co = MOFF[c] - g0
                            nc.tensor.matmul(
                                sc[0:m, co:co + w], kT[dsl, base:base + m],
                                qT[dsl, off:off + w], start=True, stop=True,
                            )
                        nc.scalar.activation(
                            praws[half][0:gm, g0:g0 + gw],
                            sc[0:gm, 0:gw], AF.Exp, scale=0.125)
                return praws

            def mask_block(hp, praws):
                # multiplicative 0/1 mask, in place. Sink group (cols
                # 0:512, 4 rows — the AV chain starter) first so AV can
                # begin early, then the rest in one wide mul per half.
                for half in range(2):
                    nc.vector.tensor_mul(
                        praws[half][0:4, 0:512], praws[half][0:4, 0:512],
                        tmask[0:4, 0:512])
                    nc.vector.tensor_mul(
                        praws[half][:, 512:MTOT], praws[half][:, 512:MTOT],
                        tmask[:, 512:MTOT])
                return praws

            def av_norm_block(hp, pts):
                # One yt bank + one dd bank per hp: half0's outputs at psum
                # partitions 0:64, half1's at 64:128 (tile_position col
                # group from out base partition), so the two halves' chains
                # run concurrently in the PE array. A DVE memset zeroes the
                # bank first; all matmuls use start=False, which is
                # order-safe: DVE writes don't touch has_written, so each
                # first matmul either accumulates onto 0 or overwrites.
                # dd is the fused denominator+broadcast: stationary
                # ones[m, 64] row-sums pt AND replicates across partitions.
                yt = psyd.tile([128, 512], f32, tag="yd", name=f"yt{hp}")
                dd = psyd.tile([128, 512], f32, tag="yd", name=f"dd{hp}")
                nc.vector.memset(yt[:], 0.0)
                nc.vector.memset(dd[:], 0.0)
                h = [hp * 2, hp * 2 + 1]
                for ci, c in enumerate(AVORD):
                    w, off, m = W_C[c], OFF_C[c], CROWS[c]
                    for half in range(2):
                        nc.tensor.matmul(
                            yt[half * 64:half * 64 + 64, off:off + w],
                            v_sb[c][0:m, h[half] * 64:(h[half] + 1) * 64],
                            pts[half][0:m, MOFF[c]:MOFF[c] + w],
                            start=False, stop=(ci == 6 and half == 1),
                            skip_group_check=True,
                        )
                for ci, c in enumerate(AVORD):
                    w, off, m = W_C[c], OFF_C[c], CROWS[c]
                    for half in range(2):
                        nc.tensor.matmul(
                            dd[half * 64:half * 64 + 64, off:off + w],
                            tones[0:m, 0:64],
                            pts[half][0:m, MOFF[c]:MOFF[c] + w],
                            start=False, stop=(ci == 6 and half == 1),
                            skip_group_check=True,
                        )
                # reciprocal_approx_fast needs SBUF f32 source (PSUM source
                # returns garbage — HW-probed), so evacuate dd via Act first
                dsb = rcpp.tile([128, 512], f32, tag="dsb", name=f"dsb{hp}")
                rcp = rcpp.tile([128, 512], f32, tag="rcp", name=f"rcp{hp}")
                ytu = pers.tile([128, CH], bf16, tag=f"ytu{hp}",
                                name=f"ytu{hp}")
                nc.scalar.copy(dsb[:], dd[:])
                nc.scalar.copy(ytu[:], yt[:])
                nc.vector.reciprocal_approx_fast(rcp[:], dsb[:])
                nc.vector.tensor_mul(ytu[:], ytu[:], rcp[:])
                return ytu

            # ---------- prologue: rope(0), V, rope(1) ----------
            qk_state = [None] * 8
            qk_state[0] = qkv_rope(0)

            v_sb = []
            for tt in range(7):
                m = CROWS[tt]
                vt = pers.tile([128, 1024], bf16, tag=f"v{tt}", name=f"v{tt}")
                pv = [psmm.tile([128, 512], f32, tag="mm", name=f"pv{tt}_{i}")
                      for i in range(2)]
                for kc in range(8):
                    if CBASE[tt] < CH:
                        xsl = xq[kc][:, CBASE[tt]:CBASE[tt] + m]
                    else:
                        xsl = xh[kc][:, CBASE[tt] - CH:CBASE[tt] - CH + m]
                    for dh in range(2):
                        nc.tensor.matmul(
                            pv[dh][0:m, :], xsl,
                            wvc[kc][:, dh * 512:(dh + 1) * 512],
                            start=(kc == 0), stop=(kc == 7),
                        )
                for dh in range(2):
                    nc.scalar.copy(
                        vt[0:m, dh * 512:(dh + 1) * 512], pv[dh][0:m, :])
                v_sb.append(vt)

            qk_state[1] = qkv_rope(1)

            # ---------- software-pipelined head-pair loop (rope depth 2).
            # The first-half projection chains (over hp 0..3) are emitted
            # inside the loop once yts[0..3] exist: they fill PE idle in
            # the rope-less tail iterations and keep HAM at K=8/8.
            yts = []
            acc_a = [None] * 8
            for hp in range(8):
                praws = sc_exp_block(hp, *qk_state[hp])
                pts = mask_block(hp, praws)
                if hp + 2 < 8:
                    qk_state[hp + 2] = qkv_rope(hp + 2)
                yts.append(av_norm_block(hp, pts))
                # rope is fully emitted by hp==5, so psmm is free of rope
                # tiles from here on: fill the rope-less tail iterations
                # with the first-half projection chains (hp 0..3).
                if 5 <= hp <= 7:
                    for cc in (2 * (hp - 5), 2 * (hp - 5) + 1):
                        po = psmm.tile([128, 512], f32, tag="mm",
                                       name=f"poa{cc}")
                        for hpp in range(4):
                            nc.tensor.matmul(
                                po[:], wpsl[cc][:, hpp * 128:(hpp + 1) * 128],
                                yts[hpp][:],
                                start=(hpp == 0), stop=(hpp == 3),
                            )
                        a = pers.tile([128, 512], f32, tag=f"acc{cc}",
                                      name=f"acc{cc}")
                        nc.vector.tensor_copy(a[:], po[:])
                        acc_a[cc] = a

            # ---------- projection second half + combine (transposed) ----
            for cc in range(8):
                po = psmm.tile([128, 512], f32, tag="mm", name=f"po{cc}")
                hp0 = 4 if cc < 6 else 0
                for hp in range(hp0, 8):
                    nc.tensor.matmul(
                        po[:], wpsl[cc][:, hp * 128:(hp + 1) * 128],
                        yts[hp][:],
                        start=(hp == hp0), stop=(hp == 7),
                    )
                osb = osbp.tile([128, 512], f32, tag="osb", name=f"osb{cc}")
                if cc < 6:
                    nc.vector.tensor_add(osb[:], po[:], acc_a[cc][:])
                else:
                    nc.vector.tensor_copy(osb[:], po[:])
                nc.sync.dma_start(outT[cc * 128:(cc + 1) * 128, :], osb[:])

    nc.compile()
    return nc


def _host_inputs(x, w_attn, w_proj):
    """Build the 8 per-core input maps (bf16 + fp8 pair layouts)."""
    import ml_dtypes
    BF = ml_dtypes.bfloat16
    E4 = ml_dtypes.float8_e4m3

    inv_freq = 1.0 / (10000.0 ** (np.arange(0, HD, 2, dtype=np.float32) / HD))
    iff = np.concatenate([inv_freq, inv_freq])  # [64]

    def cos_sin(pos):
        ang = pos[None, :].astype(np.float32) * iff[:, None]
        c = np.concatenate([np.cos(ang), np.cos(ang)], 0)
        s = np.concatenate([np.sin(ang), np.sin(ang)], 0)
        return np.ascontiguousarray(c.astype(BF)), \
            np.ascontiguousarray(s.astype(BF))

    P2 = np.zeros((128, 128), np.float32)
    for blk in range(2):
        o = blk * 64
        for d in range(32):
            P2[o + d + 32, o + d] = -1.0
            P2[o + d, o + d + 32] = 1.0
    P2 = P2.astype(BF)

    def shuffle_lhsT(w):
        # rows (kc*128 + c_lo), cols (hp*128 + d) ->
        # rows (hp*128 + c_lo), cols (kc*128 + d)
        return np.ascontiguousarray(
            w.reshape(8, 128, 8, 128).transpose(2, 1, 0, 3).reshape(C, C)
        )

    wq = shuffle_lhsT(w_attn[:, 0:C]).astype(BF)
    wk = shuffle_lhsT(w_attn[:, C:2 * C]).astype(BF)
    wvm = np.ascontiguousarray(w_attn[:, 2 * C:3 * C]).astype(BF)
    wp = shuffle_lhsT(w_proj).astype(BF)

    in_maps = []
    for core in range(NCORES):
        b, j = core // 4, core % 4
        q0 = j * CH
        kv_gk = np.full(KV, -1, np.int64)
        kv_gk[0:512] = q0 + np.arange(CH)
        halo = q0 - 256 + np.arange(256)
        kv_gk[512:768] = np.where(halo >= 0, halo, -1)
        kv_gk[768:772] = np.arange(4)

        xTc = np.zeros((C, KV), np.float32)
        valid = kv_gk >= 0
        xTc[:, valid] = x[b, kv_gk[valid]].T

        cq, sq = cos_sin(q0 + np.arange(CH))
        ck, sk = cos_sin(np.maximum(kv_gk, 0))

        gq = q0 + np.arange(CH)
        mask = np.zeros((128, MTOT), np.float32)
        for c in range(7):
            rows = CBASE[c] + np.arange(CROWS[c])
            gk = kv_gk[rows]
            qw = gq[OFF_C[c]:OFF_C[c] + W_C[c]]
            real = gk >= 0
            g = np.where(real, gk, 0)[:, None]
            qq = qw[None, :]
            if c == 6:
                allow = (g <= qq) & (qq - g >= WIN)
            else:
                allow = (g <= qq) & (qq - g < WIN)
            allow &= real[:, None]
            mask[0:CROWS[c], MOFF[c]:MOFF[c] + W_C[c]] = allow
        in_maps.append({
            "xT": np.ascontiguousarray(xTc.astype(BF)),
            "wqs": wq, "wks": wk, "wv": wvm, "wps": wp,
            "cos_q": cq, "sin_q": sq, "cos_k": ck, "sin_k": sk,
            "masks": np.ascontiguousarray(mask.astype(BF)),
            "p2": P2,
        })
    return in_maps


def kernel(x, w_attn, w_proj):
    from concourse import bass_utils

    x = np.asarray(x, np.float32)
    w_attn = np.asarray(w_attn, np.float32)
    w_proj = np.asarray(w_proj, np.float32)

    if "nc" not in _cache:
        _cache["nc"] = _build_nc()
    nc = _cache["nc"]

    in_maps = _host_inputs(x, w_attn, w_proj)
    res = bass_utils.run_bass_kernel_spmd(nc, in_maps, list(range(NCORES)),
                                          **_cache.get("run_kwargs", {}))
    _cache["last_result"] = res

    y = np.zeros((B, T, C), np.float32)
    for core in range(NCORES):
        b, j = core // 4, core % 4
        y[b, j * CH:(j + 1) * CH, :] = res.results[core]["outT"].T
    return y


# revision 24
# speedup vs baseline: 1.0494x; 1.0494x over previous
"""Trainium2 Bass kernel for CausalSelfAttention (sliding window 256 + 4 sink).

v4: 8 cores = (batch 2) x (seq chunks of 512). Per core:
KV = 772 = [own 512 | halo 256 | sink 4], QKV + RoPE in bf16, banded
attention in scores-transposed [k, q] layout with 7 key chunks.

Changes over v3:
- PE warmup matmuls right after the first x tile lands (HAM K=8/8 by the
  time real work starts).
- AV col-split: both head-halves' AV chains run concurrently in the PE
  array (half0 -> psum partitions 0:64, half1 -> 64:128 via tile_position),
  yt is ONE [128, 512] psum tile, evacuated with ONE Act copy.
- Denominator+broadcast fused: ones[m, 64] stationary -> the PE both
  row-sums pt AND replicates the result across 64 partitions; the two
  halves' chains run concurrently. No dh copies, no sel broadcast MMs.
- reciprocal_approx_fast (1 DVE op, fp32) instead of iterative RECIPROCAL
  (3.4us -> ~0.7us, off the critical path).
- masks multiplied per group (in-place into praw) so AV starts earlier.
- V stored plain [128, 1024] (no ones column, no strided copies).
- proj psum evacuation on DVE (Act is the busier engine there).
"""

import numpy as np

B, T, C, NH, HD = 2, 2048, 1024, 16, 64
WIN, SINK = 256, 4
CH = 512          # queries per core
KV = 772          # 512 own + 256 halo + 4 sink
KVP = 784         # KV padded so fp8 pair-stride is 16B-aligned (DoubleRow ISA)
NCORES = 8
W_C = [384, 384, 256, 128, 128, 256, 512]
OFF_C = [0, 128, 256, 384, 0, 0, 0]
# Score chunks packed into psum banks of 512 cols each; praw/mask share
# this column layout. Sink group first: chunk 6 is the AV starter.
GROUPS = [[6], [0, 3], [1, 4], [2, 5]]
MOFF = np.zeros(8, int)
_off = 0
for _g in GROUPS:
    for _c in _g:
        MOFF[_c] = _off
        _off += W_C[_c]
MTOT = _off  # 2048
AVORD = [6, 0, 3, 1, 4, 2, 5]
CROWS = [128, 128, 128, 128, 128, 128, 4]
CBASE = [0, 128, 256, 384, 512, 640, 768]
KSEG = [(0, 512), (512, 260)]  # kv free-dim segments (psum bank limit)
NWARM = 14

_cache = {}


def _build_nc():
    import concourse.bacc as bacc
    import concourse.mybir as mybir
    import concourse.tile as tile

    f32 = mybir.dt.float32
    bf16 = mybir.dt.bfloat16
    fp8 = mybir.dt.float8e4
    DR = mybir.MatmulPerfMode.DoubleRow
    AF = mybir.ActivationFunctionType

    nc = bacc.Bacc("TRN2", target_bir_lowering=False, debug=False,
                   num_devices=NCORES)

    xT = nc.dram_tensor("xT", [C, KV], bf16, kind="ExternalInput").ap()
    wqk = nc.dram_tensor("wqk", [C, 2 * C], bf16, kind="ExternalInput").ap()
    wv = nc.dram_tensor("wv", [C, C], bf16, kind="ExternalInput").ap()
    wps = nc.dram_tensor("wps", [C, C], bf16, kind="ExternalInput").ap()
    cos_q = nc.dram_tensor("cos_q", [128, CH], bf16, kind="ExternalInput").ap()
    sin_q = nc.dram_tensor("sin_q", [128, CH], bf16, kind="ExternalInput").ap()
    cos_k = nc.dram_tensor("cos_k", [128, KV], bf16, kind="ExternalInput").ap()
    sin_k = nc.dram_tensor("sin_k", [128, KV], bf16, kind="ExternalInput").ap()
    masks = nc.dram_tensor("masks", [128, MTOT], bf16,
                           kind="ExternalInput").ap()
    p2d = nc.dram_tensor("p2", [128, 128], bf16, kind="ExternalInput").ap()
    outT = nc.dram_tensor("outT", [C, CH], f32, kind="ExternalOutput").ap()

    with tile.TileContext(nc) as tc:
        with (
            tc.tile_pool(name="pers", bufs=1) as pers,
            tc.tile_pool(name="qk", bufs=3) as qkp,
            tc.tile_pool(name="tmp", bufs=2) as tmp,
            tc.tile_pool(name="praw", bufs=2) as prp,
            tc.tile_pool(name="rcp", bufs=2) as rcpp,
            tc.tile_pool(name="osb", bufs=2) as osbp,
            tc.tile_pool(name="psmm", bufs=2, space="PSUM") as psmm,
            tc.tile_pool(name="pssc", bufs=3, space="PSUM") as pssc,
            tc.tile_pool(name="psyd", bufs=3, space="PSUM") as psyd,
        ):
            # ones block: denominator chains use cols 0:64; also the PE
            # warmup operand (ready ~immediately, unlike any DMA)
            tones = pers.tile([128, 512], bf16, tag="ones")
            nc.gpsimd.memset(tones[:], 1.0)

            # PE warmup: HAM un-throttles after ~3.4us of sustained PE
            # activity; burn the DMA-load window on dummy matmuls so real
            # matmuls run at 2.4 GHz from the start.
            for wi in range(NWARM):
                pw = psmm.tile([128, 512], f32, tag="mm", name=f"warm{wi}")
                nc.tensor.matmul(pw[:], tones[:, 0:128], tones[:],
                                 start=True, stop=True)

            # Batched loads: the Sync engine costs ~0.6us of issue time
            # per dma_start regardless of size, so the 8-way-split tensors
            # are loaded as ONE wide tile each and sliced with views.
            HB = KV - CH
            xqall = pers.tile([128, 8, CH], bf16, tag="xqall")
            nc.sync.dma_start(
                xqall[:], xT[:, 0:CH].rearrange("(i p) k -> p i k", p=128))
            xq = [xqall[:, i, :] for i in range(8)]

            wqkt = [None] * 8

            def load_wqk(i):
                t = pers.tile([128, 2048], bf16, tag=f"wqk{i}",
                              name=f"wqk{i}")
                nc.sync.dma_start(t[:], wqk[i * 128:(i + 1) * 128, :])
                wqkt[i] = t

            load_wqk(0)
            tcos_q = pers.tile([128, CH], bf16, tag="cos_q")
            nc.sync.dma_start(tcos_q[:], cos_q[:])
            tsin_q = pers.tile([128, CH], bf16, tag="sin_q")
            nc.sync.dma_start(tsin_q[:], sin_q[:])
            tp2 = pers.tile([128, 128], bf16, tag="p2")
            nc.sync.dma_start(tp2[:], p2d[:])
            xhall = pers.tile([128, 8, HB], bf16, tag="xhall")
            nc.sync.dma_start(
                xhall[:], xT[:, CH:KV].rearrange("(i p) k -> p i k", p=128))
            xh = [xhall[:, i, :] for i in range(8)]
            tcos_k = pers.tile([128, KV], bf16, tag="cos_k")
            nc.sync.dma_start(tcos_k[:], cos_k[:])
            tsin_k = pers.tile([128, KV], bf16, tag="sin_k")
            nc.sync.dma_start(tsin_k[:], sin_k[:])
            wvall = pers.tile([128, 8, 1024], bf16, tag="wvall")
            nc.sync.dma_start(
                wvall[:], wv.rearrange("(i p) n -> p i n", p=128))
            wvc = [wvall[:, i, :] for i in range(8)]
            load_wqk(1)
            tmask = pers.tile([128, MTOT], bf16, tag="mask")
            nc.sync.dma_start(tmask[:], masks[:])
            for i in range(2, 8):
                load_wqk(i)
            wpall = pers.tile([128, 8, 1024], bf16, tag="wpall")
            nc.sync.dma_start(
                wpall[:], wps.rearrange("(i p) n -> p i n", p=128))
            wpsl = [wpall[:, i, :] for i in range(8)]

            def qkv_rope(hp):
                # qT raw -> rope
                pq = psmm.tile([128, 512], f32, tag="mm", name=f"pq{hp}")
                for kc in range(8):
                    nc.tensor.matmul(
                        pq[:], wqkt[hp][:, kc * 128:(kc + 1) * 128],
                        xq[kc][:],
                        start=(kc == 0), stop=(kc == 7),
                    )
                qraw = tmp.tile([128, CH], bf16, tag="qraw", name=f"qraw{hp}")
                nc.scalar.copy(qraw[:], pq[:])
                qT = qkp.tile([128, CH], bf16, tag="qT", name=f"qT{hp}")
                prot = psmm.tile([128, 512], f32, tag="mm", name=f"prot{hp}")
                nc.tensor.matmul(prot[:], tp2[:], qraw[:], start=True,
                                 stop=True)
                t2 = tmp.tile([128, CH], bf16, tag="t2", name=f"t2q{hp}")
                nc.vector.tensor_mul(t2[:], prot[:], tsin_q[:])
                qm = tmp.tile([128, CH], bf16, tag="qm", name=f"qm{hp}")
                nc.vector.tensor_mul(qm[:], qraw[:], tcos_q[:])
                nc.vector.tensor_add(qT[:], qm[:], t2[:])

                # kT raw -> rope (two psum segments)
                kraw = tmp.tile([128, KV], bf16, tag="kraw", name=f"kraw{hp}")
                pk = [psmm.tile([128, 512], f32, tag="mm", name=f"pk{hp}_{i}")
                      for i in range(2)]
                for kc in range(8):
                    for si, (s0, sw) in enumerate(KSEG):
                        rhs = xq[kc][:] if si == 0 else xh[kc][:, 0:sw]
                        nc.tensor.matmul(
                            pk[si][:, 0:sw],
                            wqkt[hp][:, 1024 + kc * 128:1024 + (kc + 1) * 128],
                            rhs,
                            start=(kc == 0), stop=(kc == 7),
                        )
                kT = qkp.tile([128, KV], bf16, tag="kT", name=f"kT{hp}")
                for si, (s0, sw) in enumerate(KSEG):
                    nc.scalar.copy(kraw[:, s0:s0 + sw], pk[si][:, 0:sw])
                    prk = psmm.tile([128, 512], f32, tag="mm",
                                    name=f"prk{hp}_{si}")
                    nc.tensor.matmul(prk[:, 0:sw], tp2[:],
                                     kraw[:, s0:s0 + sw], start=True,
                                     stop=True)
                    t2k = tmp.tile([128, 512], bf16, tag="t2",
                                   name=f"t2k{hp}_{si}")
                    nc.vector.tensor_mul(t2k[:, 0:sw], prk[:, 0:sw],
                                         tsin_k[:, s0:s0 + sw])
                    km = tmp.tile([128, 512], bf16, tag="qm",
                                  name=f"km{hp}_{si}")
                    nc.vector.tensor_mul(km[:, 0:sw], kraw[:, s0:s0 + sw],
                                         tcos_k[:, s0:s0 + sw])
                    nc.vector.tensor_add(kT[:, s0:s0 + sw], km[:, 0:sw],
                                         t2k[:, 0:sw])
                return qT, kT

            def sc_exp_block(hp, qT, kT):
                # scoresT: per chunk, two head-halves at partition bases 0/64
                # run concurrently in the PE array. Chunks within a GROUP
                # share one 512-wide psum bank, so each (group, half) needs
                # only ONE exp (fused 1/8 scale) into the contiguous
                # [128, MTOT] praw tile per half.
                praws = []
                for half in range(2):
                    praws.append(prp.tile([128, MTOT], bf16, tag="praw",
                                          name=f"praw{hp}_{half}"))
                for gi, grp in enumerate(GROUPS):
                    g0 = MOFF[grp[0]]
                    gw = sum(W_C[c] for c in grp)
                    gm = max(CROWS[c] for c in grp)
                    for half in range(2):
                        dsl = slice(half * 64, half * 64 + 64)
                        sc = pssc.tile([128, 512], f32, tag="sc",
                                       name=f"sc{hp}_{gi}_{half}")
                        for c in grp:
                            w, off, m = W_C[c], OFF_C[c], CROWS[c]
                            base = CBASE[c]
                            co = MOFF[c] - g0
                            nc.tensor.matmul(
                                sc[0:m, co:co + w], kT[dsl, base:base + m],
                                qT[dsl, off:off + w], start=True, stop=True,
                            )
                        nc.scalar.activation(
                            praws[half][0:gm, g0:g0 + gw],
                            sc[0:gm, 0:gw], AF.Exp, scale=0.125)
                return praws

            def mask_block(hp, praws):
                # multiplicative 0/1 mask, in place. Sink group (cols
                # 0:512, 4 rows — the AV chain starter) first so AV can
                # begin early, then the rest in one wide mul per half.
                for half in range(2):
                    nc.vector.tensor_mul(
                        praws[half][0:4, 0:512], praws[half][0:4, 0:512],
                        tmask[0:4, 0:512])
                    nc.vector.tensor_mul(
                        praws[half][:, 512:MTOT], praws[half][:, 512:MTOT],
                        tmask[:, 512:MTOT])
                return praws

            def av_alloc(hp):
                # yt/dd banks are DVE-memset to zero well before the AV
                # chains need them (the memset must not sit on the AV
                # critical path); all AV/dd matmuls then use start=False,
                # which is order-safe: DVE writes don't touch has_written,
                # so each first matmul either accumulates onto 0 or
                # overwrites.
                yt = psyd.tile([128, 512], f32, tag="yd", name=f"yt{hp}")
                dd = psyd.tile([128, 512], f32, tag="yd", name=f"dd{hp}")
                nc.vector.memset(yt[:], 0.0)
                nc.vector.memset(dd[:], 0.0)
                return yt, dd

            def av_norm_block(hp, pts, yt, dd):
                # One yt bank + one dd bank per hp: half0's outputs at psum
                # partitions 0:64, half1's at 64:128 (tile_position col
                # group from out base partition), so the two halves' chains
                # run concurrently in the PE array. dd is the fused
                # denominator+broadcast: stationary ones[m, 64] row-sums
                # pt AND replicates across partitions.
                h = [hp * 2, hp * 2 + 1]
                for ci, c in enumerate(AVORD):
                    w, off, m = W_C[c], OFF_C[c], CROWS[c]
                    for half in range(2):
                        nc.tensor.matmul(
                            yt[half * 64:half * 64 + 64, off:off + w],
                            v_sb[c][0:m, h[half] * 64:(h[half] + 1) * 64],
                            pts[half][0:m, MOFF[c]:MOFF[c] + w],
                            start=False, stop=(ci == 6 and half == 1),
                            skip_group_check=True,
                        )
                for ci, c in enumerate(AVORD):
                    w, off, m = W_C[c], OFF_C[c], CROWS[c]
                    for half in range(2):
                        nc.tensor.matmul(
                            dd[half * 64:half * 64 + 64, off:off + w],
                            tones[0:m, 0:64],
                            pts[half][0:m, MOFF[c]:MOFF[c] + w],
                            start=False, stop=(ci == 6 and half == 1),
                            skip_group_check=True,
                        )
                # reciprocal_approx_fast needs SBUF f32 source (PSUM source
                # returns garbage — HW-probed), so evacuate dd via Act first
                dsb = rcpp.tile([128, 512], f32, tag="dsb", name=f"dsb{hp}")
                rcp = rcpp.tile([128, 512], f32, tag="rcp", name=f"rcp{hp}")
                ytu = pers.tile([128, CH], bf16, tag=f"ytu{hp}",
                                name=f"ytu{hp}")
                nc.scalar.copy(dsb[:], dd[:])
                nc.scalar.copy(ytu[:], yt[:])
                nc.vector.reciprocal_approx_fast(rcp[:], dsb[:])
                nc.vector.tensor_mul(ytu[:], ytu[:], rcp[:])
                return ytu

            # ---------- prologue: rope(0), V, rope(1) ----------
            qk_state = [None] * 8
            qk_state[0] = qkv_rope(0)

            v_sb = []
            for tt in range(7):
                m = CROWS[tt]
                vt = pers.tile([128, 1024], bf16, tag=f"v{tt}", name=f"v{tt}")
                pv = [psmm.tile([128, 512], f32, tag="mm", name=f"pv{tt}_{i}")
                      for i in range(2)]
                for kc in range(8):
                    if CBASE[tt] < CH:
                        xsl = xq[kc][:, CBASE[tt]:CBASE[tt] + m]
                    else:
                        xsl = xh[kc][:, CBASE[tt] - CH:CBASE[tt] - CH + m]
                    for dh in range(2):
                        nc.tensor.matmul(
                            pv[dh][0:m, :], xsl,
                            wvc[kc][:, dh * 512:(dh + 1) * 512],
                            start=(kc == 0), stop=(kc == 7),
                        )
                for dh in range(2):
                    nc.scalar.copy(
                        vt[0:m, dh * 512:(dh + 1) * 512], pv[dh][0:m, :])
                v_sb.append(vt)

            qk_state[1] = qkv_rope(1)

            # ---------- software-pipelined head-pair loop (rope depth 2).
            # The first-half projection chains (over hp 0..3) are emitted
            # inside the loop once yts[0..3] exist: they fill PE idle in
            # the rope-less tail iterations and keep HAM at K=8/8.
            yts = []
            acc_a = [None] * 8
            for hp in range(8):
                ytdd = av_alloc(hp)
                praws = sc_exp_block(hp, *qk_state[hp])
                pts = mask_block(hp, praws)
                if hp + 2 < 8:
                    qk_state[hp + 2] = qkv_rope(hp + 2)
                yts.append(av_norm_block(hp, pts, *ytdd))
                # rope is fully emitted by hp==5, so psmm is free of rope
                # tiles from here on: fill the rope-less tail iterations
                # with the first-half projection chains (hp 0..3).
                if 5 <= hp <= 7:
                    for cc in (2 * (hp - 5), 2 * (hp - 5) + 1):
                        po = psmm.tile([128, 512], f32, tag="mm",
                                       name=f"poa{cc}")
                        for hpp in range(4):
                            nc.tensor.matmul(
                                po[:], wpsl[cc][:, hpp * 128:(hpp + 1) * 128],
                                yts[hpp][:],
                                start=(hpp == 0), stop=(hpp == 3),
                            )
                        a = pers.tile([128, 512], f32, tag=f"acc{cc}",
                                      name=f"acc{cc}")
                        nc.vector.tensor_copy(a[:], po[:])
                        acc_a[cc] = a

            # ---------- projection second half + combine (transposed) ----
            for cc in range(8):
                po = psmm.tile([128, 512], f32, tag="mm", name=f"po{cc}")
                hp0 = 4 if cc < 6 else 0
                for hp in range(hp0, 8):
                    nc.tensor.matmul(
                        po[:], wpsl[cc][:, hp * 128:(hp + 1) * 128],
                        yts[hp][:],
                        start=(hp == hp0), stop=(hp == 7),
                    )
                osb = osbp.tile([128, 512], f32, tag="osb", name=f"osb{cc}")
                if cc < 6:
                    nc.vector.tensor_add(osb[:], po[:], acc_a[cc][:])
                else:
                    nc.vector.tensor_copy(osb[:], po[:])
                nc.sync.dma_start(outT[cc * 128:(cc + 1) * 128, :], osb[:])

    nc.compile()
    return nc


def _host_inputs(x, w_attn, w_proj):
    """Build the 8 per-core input maps (bf16 + fp8 pair layouts)."""
    import ml_dtypes
    BF = ml_dtypes.bfloat16
    E4 = ml_dtypes.float8_e4m3

    inv_freq = 1.0 / (10000.0 ** (np.arange(0, HD, 2, dtype=np.float32) / HD))
    iff = np.concatenate([inv_freq, inv_freq])  # [64]

    def cos_sin(pos):
        ang = pos[None, :].astype(np.float32) * iff[:, None]
        c = np.concatenate([np.cos(ang), np.cos(ang)], 0)
        s = np.concatenate([np.sin(ang), np.sin(ang)], 0)
        return np.ascontiguousarray(c.astype(BF)), \
            np.ascontiguousarray(s.astype(BF))

    P2 = np.zeros((128, 128), np.float32)
    for blk in range(2):
        o = blk * 64
        for d in range(32):
            P2[o + d + 32, o + d] = -1.0
            P2[o + d, o + d + 32] = 1.0
    P2 = P2.astype(BF)

    def shuffle_lhsT(w):
        # rows (kc*128 + c_lo), cols (hp*128 + d) ->
        # rows (hp*128 + c_lo), cols (kc*128 + d)
        return np.ascontiguousarray(
            w.reshape(8, 128, 8, 128).transpose(2, 1, 0, 3).reshape(C, C)
        )

    wq = shuffle_lhsT(w_attn[:, 0:C]).astype(BF)
    wk = shuffle_lhsT(w_attn[:, C:2 * C]).astype(BF)
    wqkm = np.ascontiguousarray(np.concatenate([wq, wk], axis=1))
    wvm = np.ascontiguousarray(w_attn[:, 2 * C:3 * C]).astype(BF)
    wp = shuffle_lhsT(w_proj).astype(BF)

    in_maps = []
    for core in range(NCORES):
        b, j = core // 4, core % 4
        q0 = j * CH
        kv_gk = np.full(KV, -1, np.int64)
        kv_gk[0:512] = q0 + np.arange(CH)
        halo = q0 - 256 + np.arange(256)
        kv_gk[512:768] = np.where(halo >= 0, halo, -1)
        kv_gk[768:772] = np.arange(4)

        xTc = np.zeros((C, KV), np.float32)
        valid = kv_gk >= 0
        xTc[:, valid] = x[b, kv_gk[valid]].T

        cq, sq = cos_sin(q0 + np.arange(CH))
        ck, sk = cos_sin(np.maximum(kv_gk, 0))

        gq = q0 + np.arange(CH)
        mask = np.zeros((128, MTOT), np.float32)
        for c in range(7):
            rows = CBASE[c] + np.arange(CROWS[c])
            gk = kv_gk[rows]
            qw = gq[OFF_C[c]:OFF_C[c] + W_C[c]]
            real = gk >= 0
            g = np.where(real, gk, 0)[:, None]
            qq = qw[None, :]
            if c == 6:
                allow = (g <= qq) & (qq - g >= WIN)
            else:
                allow = (g <= qq) & (qq - g < WIN)
            allow &= real[:, None]
            mask[0:CROWS[c], MOFF[c]:MOFF[c] + W_C[c]] = allow
        in_maps.append({
            "xT": np.ascontiguousarray(xTc.astype(BF)),
            "wqk": wqkm, "wv": wvm, "wps": wp,
            "cos_q": cq, "sin_q": sq, "cos_k": ck, "sin_k": sk,
            "masks": np.ascontiguousarray(mask.astype(BF)),
            "p2": P2,
        })
    return in_maps


def kernel(x, w_attn, w_proj):
    from concourse import bass_utils

    x = np.asarray(x, np.float32)
    w_attn = np.asarray(w_attn, np.float32)
    w_proj = np.asarray(w_proj, np.float32)

    if "nc" not in _cache:
        _cache["nc"] = _build_nc()
    nc = _cache["nc"]

    in_maps = _host_inputs(x, w_attn, w_proj)
    res = bass_utils.run_bass_kernel_spmd(nc, in_maps, list(range(NCORES)),
                                          **_cache.get("run_kwargs", {}))
    _cache["last_result"] = res

    y = np.zeros((B, T, C), np.float32)
    for core in range(NCORES):
        b, j = core // 4, core % 4
        y[b, j * CH:(j + 1) * CH, :] = res.results[core]["outT"].T
    return y


# revision 25
# speedup vs baseline: 1.0888x; 1.0375x over previous
"""Trainium2 Bass kernel for CausalSelfAttention (sliding window 256 + 4 sink).

v4: 8 cores = (batch 2) x (seq chunks of 512). Per core:
KV = 772 = [own 512 | halo 256 | sink 4], QKV + RoPE in bf16, banded
attention in scores-transposed [k, q] layout with 7 key chunks.

Changes over v3:
- PE warmup matmuls right after the first x tile lands (HAM K=8/8 by the
  time real work starts).
- AV col-split: both head-halves' AV chains run concurrently in the PE
  array (half0 -> psum partitions 0:64, half1 -> 64:128 via tile_position),
  yt is ONE [128, 512] psum tile, evacuated with ONE Act copy.
- Denominator+broadcast fused: ones[m, 64] stationary -> the PE both
  row-sums pt AND replicates the result across 64 partitions; the two
  halves' chains run concurrently. No dh copies, no sel broadcast MMs.
- reciprocal_approx_fast (1 DVE op, fp32) instead of iterative RECIPROCAL
  (3.4us -> ~0.7us, off the critical path).
- masks multiplied per group (in-place into praw) so AV starts earlier.
- V stored plain [128, 1024] (no ones column, no strided copies).
- proj psum evacuation on DVE (Act is the busier engine there).
"""

import numpy as np

B, T, C, NH, HD = 2, 2048, 1024, 16, 64
WIN, SINK = 256, 4
CH = 512          # queries per core
KV = 772          # 512 own + 256 halo + 4 sink
KVP = 784         # KV padded so fp8 pair-stride is 16B-aligned (DoubleRow ISA)
NCORES = 8
W_C = [384, 384, 256, 128, 128, 256, 512]
OFF_C = [0, 128, 256, 384, 0, 0, 0]
# Score chunks packed into psum banks of 512 cols each; praw/mask share
# this column layout. Sink group first: chunk 6 is the AV starter.
GROUPS = [[6], [0, 3], [1, 4], [2, 5]]
MOFF = np.zeros(8, int)
_off = 0
for _g in GROUPS:
    for _c in _g:
        MOFF[_c] = _off
        _off += W_C[_c]
MTOT = _off  # 2048
AVORD = [6, 0, 3, 1, 4, 2, 5]
CROWS = [128, 128, 128, 128, 128, 128, 4]
CBASE = [0, 128, 256, 384, 512, 640, 768]
KSEG = [(0, 512), (512, 260)]  # kv free-dim segments (psum bank limit)
NWARM = 14

_cache = {}


def _build_nc():
    import concourse.bacc as bacc
    import concourse.mybir as mybir
    import concourse.tile as tile

    f32 = mybir.dt.float32
    bf16 = mybir.dt.bfloat16
    fp8 = mybir.dt.float8e4
    DR = mybir.MatmulPerfMode.DoubleRow
    AF = mybir.ActivationFunctionType

    nc = bacc.Bacc("TRN2", target_bir_lowering=False, debug=False,
                   num_devices=NCORES)

    xT = nc.dram_tensor("xT", [C, KV], bf16, kind="ExternalInput").ap()
    wqk = nc.dram_tensor("wqk", [C, 2 * C], bf16, kind="ExternalInput").ap()
    wv = nc.dram_tensor("wv", [C, C], bf16, kind="ExternalInput").ap()
    wps = nc.dram_tensor("wps", [C, C], bf16, kind="ExternalInput").ap()
    cos_q = nc.dram_tensor("cos_q", [128, CH], bf16, kind="ExternalInput").ap()
    sin_q = nc.dram_tensor("sin_q", [128, CH], bf16, kind="ExternalInput").ap()
    cos_k = nc.dram_tensor("cos_k", [128, KV], bf16, kind="ExternalInput").ap()
    sin_k = nc.dram_tensor("sin_k", [128, KV], bf16, kind="ExternalInput").ap()
    masks = nc.dram_tensor("masks", [128, MTOT], bf16,
                           kind="ExternalInput").ap()
    p2d = nc.dram_tensor("p2", [128, 128], bf16, kind="ExternalInput").ap()
    outT = nc.dram_tensor("outT", [C, CH], f32, kind="ExternalOutput").ap()

    with tile.TileContext(nc) as tc:
        with (
            tc.tile_pool(name="pers", bufs=1) as pers,
            tc.tile_pool(name="qk", bufs=3) as qkp,
            tc.tile_pool(name="tmp", bufs=2) as tmp,
            tc.tile_pool(name="praw", bufs=2) as prp,
            tc.tile_pool(name="rcp", bufs=2) as rcpp,
            tc.tile_pool(name="osb", bufs=2) as osbp,
            tc.tile_pool(name="psmm", bufs=2, space="PSUM") as psmm,
            tc.tile_pool(name="pssc", bufs=3, space="PSUM") as pssc,
            tc.tile_pool(name="psyd", bufs=3, space="PSUM") as psyd,
        ):
            # ones block: denominator chains use cols 0:64; also the PE
            # warmup operand (ready ~immediately, unlike any DMA)
            tones = pers.tile([128, 512], bf16, tag="ones")
            nc.gpsimd.memset(tones[:], 1.0)

            # PE warmup: HAM un-throttles after ~3.4us of sustained PE
            # activity; burn the DMA-load window on dummy matmuls so real
            # matmuls run at 2.4 GHz from the start.
            for wi in range(NWARM):
                pw = psmm.tile([128, 512], f32, tag="mm", name=f"warm{wi}")
                nc.tensor.matmul(pw[:], tones[:, 0:128], tones[:],
                                 start=True, stop=True)

            # Batched loads: the Sync engine costs ~0.6us of issue time
            # per dma_start regardless of size, so the 8-way-split tensors
            # are loaded as ONE wide tile each and sliced with views.
            HB = KV - CH
            xqall = pers.tile([128, 8, CH], bf16, tag="xqall")
            nc.sync.dma_start(
                xqall[:], xT[:, 0:CH].rearrange("(i p) k -> p i k", p=128))
            xq = [xqall[:, i, :] for i in range(8)]

            wqkt = [None] * 8

            def load_wqk(i):
                t = pers.tile([128, 2048], bf16, tag=f"wqk{i}",
                              name=f"wqk{i}")
                nc.sync.dma_start(t[:], wqk[i * 128:(i + 1) * 128, :])
                wqkt[i] = t

            load_wqk(0)
            tcos_q = pers.tile([128, CH], bf16, tag="cos_q")
            nc.sync.dma_start(tcos_q[:], cos_q[:])
            tsin_q = pers.tile([128, CH], bf16, tag="sin_q")
            nc.sync.dma_start(tsin_q[:], sin_q[:])
            tp2 = pers.tile([128, 128], bf16, tag="p2")
            nc.sync.dma_start(tp2[:], p2d[:])
            xhall = pers.tile([128, 8, HB], bf16, tag="xhall")
            nc.sync.dma_start(
                xhall[:], xT[:, CH:KV].rearrange("(i p) k -> p i k", p=128))
            xh = [xhall[:, i, :] for i in range(8)]
            tcos_k = pers.tile([128, KV], bf16, tag="cos_k")
            nc.sync.dma_start(tcos_k[:], cos_k[:])
            tsin_k = pers.tile([128, KV], bf16, tag="sin_k")
            nc.sync.dma_start(tsin_k[:], sin_k[:])
            wvall = pers.tile([128, 8, 1024], bf16, tag="wvall")
            nc.sync.dma_start(
                wvall[:], wv.rearrange("(i p) n -> p i n", p=128))
            wvc = [wvall[:, i, :] for i in range(8)]
            load_wqk(1)
            tmask = pers.tile([128, MTOT], bf16, tag="mask")
            nc.sync.dma_start(tmask[:], masks[:])
            for i in range(2, 8):
                load_wqk(i)
            wpall = pers.tile([128, 8, 1024], bf16, tag="wpall")
            nc.sync.dma_start(
                wpall[:], wps.rearrange("(i p) n -> p i n", p=128))
            wpsl = [wpall[:, i, :] for i in range(8)]

            def qkv_rope(hp):
                # qT raw -> rope
                pq = psmm.tile([128, 512], f32, tag="mm", name=f"pq{hp}")
                for kc in range(8):
                    nc.tensor.matmul(
                        pq[:], wqkt[hp][:, kc * 128:(kc + 1) * 128],
                        xq[kc][:],
                        start=(kc == 0), stop=(kc == 7),
                    )
                qraw = tmp.tile([128, CH], bf16, tag="qraw", name=f"qraw{hp}")
                nc.scalar.copy(qraw[:], pq[:])
                qT = qkp.tile([128, CH], bf16, tag="qT", name=f"qT{hp}")
                prot = psmm.tile([128, 512], f32, tag="mm", name=f"prot{hp}")
                nc.tensor.matmul(prot[:], tp2[:], qraw[:], start=True,
                                 stop=True)
                t2 = tmp.tile([128, CH], bf16, tag="t2", name=f"t2q{hp}")
                nc.vector.tensor_mul(t2[:], prot[:], tsin_q[:])
                qm = tmp.tile([128, CH], bf16, tag="qm", name=f"qm{hp}")
                nc.vector.tensor_mul(qm[:], qraw[:], tcos_q[:])
                nc.vector.tensor_add(qT[:], qm[:], t2[:])

                # kT raw -> rope (two psum segments)
                kraw = tmp.tile([128, KV], bf16, tag="kraw", name=f"kraw{hp}")
                pk = [psmm.tile([128, 512], f32, tag="mm", name=f"pk{hp}_{i}")
                      for i in range(2)]
                for kc in range(8):
                    for si, (s0, sw) in enumerate(KSEG):
                        rhs = xq[kc][:] if si == 0 else xh[kc][:, 0:sw]
                        nc.tensor.matmul(
                            pk[si][:, 0:sw],
                            wqkt[hp][:, 1024 + kc * 128:1024 + (kc + 1) * 128],
                            rhs,
                            start=(kc == 0), stop=(kc == 7),
                        )
                kT = qkp.tile([128, KV], bf16, tag="kT", name=f"kT{hp}")
                for si, (s0, sw) in enumerate(KSEG):
                    nc.scalar.copy(kraw[:, s0:s0 + sw], pk[si][:, 0:sw])
                    prk = psmm.tile([128, 512], f32, tag="mm",
                                    name=f"prk{hp}_{si}")
                    nc.tensor.matmul(prk[:, 0:sw], tp2[:],
                                     kraw[:, s0:s0 + sw], start=True,
                                     stop=True)
                    t2k = tmp.tile([128, 512], bf16, tag="t2",
                                   name=f"t2k{hp}_{si}")
                    nc.vector.tensor_mul(t2k[:, 0:sw], prk[:, 0:sw],
                                         tsin_k[:, s0:s0 + sw])
                    km = tmp.tile([128, 512], bf16, tag="qm",
                                  name=f"km{hp}_{si}")
                    nc.vector.tensor_mul(km[:, 0:sw], kraw[:, s0:s0 + sw],
                                         tcos_k[:, s0:s0 + sw])
                    nc.vector.tensor_add(kT[:, s0:s0 + sw], km[:, 0:sw],
                                         t2k[:, 0:sw])
                return qT, kT

            def sc_exp_block(hp, qT, kT):
                # scoresT: per chunk, two head-halves at partition bases 0/64
                # run concurrently in the PE array. Chunks within a GROUP
                # share one 512-wide psum bank, so each (group, half) needs
                # only ONE exp (fused 1/8 scale) into the contiguous
                # [128, MTOT] praw tile per half.
                praws = []
                for half in range(2):
                    praws.append(prp.tile([128, MTOT], bf16, tag="praw",
                                          name=f"praw{hp}_{half}"))
                for gi, grp in enumerate(GROUPS):
                    g0 = MOFF[grp[0]]
                    gw = sum(W_C[c] for c in grp)
                    gm = max(CROWS[c] for c in grp)
                    for half in range(2):
                        dsl = slice(half * 64, half * 64 + 64)
                        sc = pssc.tile([128, 512], f32, tag="sc",
                                       name=f"sc{hp}_{gi}_{half}")
                        for c in grp:
                            w, off, m = W_C[c], OFF_C[c], CROWS[c]
                            base = CBASE[c]
                            co = MOFF[c] - g0
                            nc.tensor.matmul(
                                sc[0:m, co:co + w], kT[dsl, base:base + m],
                                qT[dsl, off:off + w], start=True, stop=True,
                            )
                        nc.scalar.activation(
                            praws[half][0:gm, g0:g0 + gw],
                            sc[0:gm, 0:gw], AF.Exp, scale=0.125)
                return praws

            def mask_block(hp, praws):
                # multiplicative 0/1 mask, in place, two muls per half
                # aligned with the AV chunk order: cols 0:1024 covers
                # groups g0+g1 (AV chunks 6,0,3) so their AV matmuls gate
                # only on the first two exps; cols 1024:2048 covers g2+g3.
                for half in range(2):
                    nc.vector.tensor_mul(
                        praws[half][:, 0:1024], praws[half][:, 0:1024],
                        tmask[:, 0:1024])
                    nc.vector.tensor_mul(
                        praws[half][:, 1024:MTOT], praws[half][:, 1024:MTOT],
                        tmask[:, 1024:MTOT])
                return praws

            def av_alloc(hp):
                # yt/dd banks are DVE-memset to zero well before the AV
                # chains need them (the memset must not sit on the AV
                # critical path); all AV/dd matmuls then use start=False,
                # which is order-safe: DVE writes don't touch has_written,
                # so each first matmul either accumulates onto 0 or
                # overwrites.
                yt = psyd.tile([128, 512], f32, tag="yd", name=f"yt{hp}")
                dd = psyd.tile([128, 512], f32, tag="yd", name=f"dd{hp}")
                nc.vector.memset(yt[:], 0.0)
                nc.vector.memset(dd[:], 0.0)
                return yt, dd

            def av_norm_block(hp, pts, yt, dd):
                # One yt bank + one dd bank per hp: half0's outputs at psum
                # partitions 0:64, half1's at 64:128 (tile_position col
                # group from out base partition), so the two halves' chains
                # run concurrently in the PE array. dd is the fused
                # denominator+broadcast: stationary ones[m, 64] row-sums
                # pt AND replicates across partitions.
                h = [hp * 2, hp * 2 + 1]
                for ci, c in enumerate(AVORD):
                    w, off, m = W_C[c], OFF_C[c], CROWS[c]
                    for half in range(2):
                        nc.tensor.matmul(
                            yt[half * 64:half * 64 + 64, off:off + w],
                            v_sb[c][0:m, h[half] * 64:(h[half] + 1) * 64],
                            pts[half][0:m, MOFF[c]:MOFF[c] + w],
                            start=False, stop=(ci == 6 and half == 1),
                            skip_group_check=True,
                        )
                for ci, c in enumerate(AVORD):
                    w, off, m = W_C[c], OFF_C[c], CROWS[c]
                    for half in range(2):
                        nc.tensor.matmul(
                            dd[half * 64:half * 64 + 64, off:off + w],
                            tones[0:m, 0:64],
                            pts[half][0:m, MOFF[c]:MOFF[c] + w],
                            start=False, stop=(ci == 6 and half == 1),
                            skip_group_check=True,
                        )
                # reciprocal_approx_fast needs SBUF f32 source (PSUM source
                # returns garbage — HW-probed), so evacuate dd via Act first
                dsb = rcpp.tile([128, 512], f32, tag="dsb", name=f"dsb{hp}")
                rcp = rcpp.tile([128, 512], f32, tag="rcp", name=f"rcp{hp}")
                ytu = pers.tile([128, CH], bf16, tag=f"ytu{hp}",
                                name=f"ytu{hp}")
                nc.scalar.copy(dsb[:], dd[:])
                nc.vector.reciprocal_approx_fast(rcp[:], dsb[:])
                nc.vector.tensor_mul(ytu[:], yt[:], rcp[:])
                return ytu

            # ---------- prologue: rope(0), V, rope(1) ----------
            qk_state = [None] * 8
            qk_state[0] = qkv_rope(0)

            v_sb = []
            for tt in range(7):
                m = CROWS[tt]
                vt = pers.tile([128, 1024], bf16, tag=f"v{tt}", name=f"v{tt}")
                pv = [psmm.tile([128, 512], f32, tag="mm", name=f"pv{tt}_{i}")
                      for i in range(2)]
                for kc in range(8):
                    if CBASE[tt] < CH:
                        xsl = xq[kc][:, CBASE[tt]:CBASE[tt] + m]
                    else:
                        xsl = xh[kc][:, CBASE[tt] - CH:CBASE[tt] - CH + m]
                    for dh in range(2):
                        nc.tensor.matmul(
                            pv[dh][0:m, :], xsl,
                            wvc[kc][:, dh * 512:(dh + 1) * 512],
                            start=(kc == 0), stop=(kc == 7),
                        )
                for dh in range(2):
                    nc.scalar.copy(
                        vt[0:m, dh * 512:(dh + 1) * 512], pv[dh][0:m, :])
                v_sb.append(vt)

            qk_state[1] = qkv_rope(1)

            # ---------- software-pipelined head-pair loop (rope depth 2).
            # The first-half projection chains (over hp 0..3) are emitted
            # inside the loop once yts[0..3] exist: they fill PE idle in
            # the rope-less tail iterations and keep HAM at K=8/8.
            yts = []
            acc_a = [None] * 8
            for hp in range(8):
                ytdd = av_alloc(hp)
                praws = sc_exp_block(hp, *qk_state[hp])
                pts = mask_block(hp, praws)
                if hp + 2 < 8:
                    qk_state[hp + 2] = qkv_rope(hp + 2)
                yts.append(av_norm_block(hp, pts, *ytdd))
                # rope is fully emitted by hp==5, so psmm is free of rope
                # tiles from here on: fill the rope-less tail iterations
                # with the first-half projection chains (hp 0..3).
                if 5 <= hp <= 7:
                    for cc in (2 * (hp - 5), 2 * (hp - 5) + 1):
                        po = psmm.tile([128, 512], f32, tag="mm",
                                       name=f"poa{cc}")
                        for hpp in range(4):
                            nc.tensor.matmul(
                                po[:], wpsl[cc][:, hpp * 128:(hpp + 1) * 128],
                                yts[hpp][:],
                                start=(hpp == 0), stop=(hpp == 3),
                            )
                        a = pers.tile([128, 512], f32, tag=f"acc{cc}",
                                      name=f"acc{cc}")
                        nc.vector.tensor_copy(a[:], po[:])
                        acc_a[cc] = a

            # ---------- projection second half + combine (transposed) ----
            for cc in range(8):
                po = psmm.tile([128, 512], f32, tag="mm", name=f"po{cc}")
                hp0 = 4 if cc < 6 else 0
                for hp in range(hp0, 8):
                    nc.tensor.matmul(
                        po[:], wpsl[cc][:, hp * 128:(hp + 1) * 128],
                        yts[hp][:],
                        start=(hp == hp0), stop=(hp == 7),
                    )
                osb = osbp.tile([128, 512], f32, tag="osb", name=f"osb{cc}")
                if cc < 6:
                    nc.vector.tensor_add(osb[:], po[:], acc_a[cc][:])
                else:
                    nc.vector.tensor_copy(osb[:], po[:])
                nc.sync.dma_start(outT[cc * 128:(cc + 1) * 128, :], osb[:])

    nc.compile()
    return nc


def _host_inputs(x, w_attn, w_proj):
    """Build the 8 per-core input maps (bf16 + fp8 pair layouts)."""
    import ml_dtypes
    BF = ml_dtypes.bfloat16
    E4 = ml_dtypes.float8_e4m3

    inv_freq = 1.0 / (10000.0 ** (np.arange(0, HD, 2, dtype=np.float32) / HD))
    iff = np.concatenate([inv_freq, inv_freq])  # [64]

    def cos_sin(pos):
        ang = pos[None, :].astype(np.float32) * iff[:, None]
        c = np.concatenate([np.cos(ang), np.cos(ang)], 0)
        s = np.concatenate([np.sin(ang), np.sin(ang)], 0)
        return np.ascontiguousarray(c.astype(BF)), \
            np.ascontiguousarray(s.astype(BF))

    P2 = np.zeros((128, 128), np.float32)
    for blk in range(2):
        o = blk * 64
        for d in range(32):
            P2[o + d + 32, o + d] = -1.0
            P2[o + d, o + d + 32] = 1.0
    P2 = P2.astype(BF)

    def shuffle_lhsT(w):
        # rows (kc*128 + c_lo), cols (hp*128 + d) ->
        # rows (hp*128 + c_lo), cols (kc*128 + d)
        return np.ascontiguousarray(
            w.reshape(8, 128, 8, 128).transpose(2, 1, 0, 3).reshape(C, C)
        )

    wq = shuffle_lhsT(w_attn[:, 0:C]).astype(BF)
    wk = shuffle_lhsT(w_attn[:, C:2 * C]).astype(BF)
    wqkm = np.ascontiguousarray(np.concatenate([wq, wk], axis=1))
    wvm = np.ascontiguousarray(w_attn[:, 2 * C:3 * C]).astype(BF)
    wp = shuffle_lhsT(w_proj).astype(BF)

    in_maps = []
    for core in range(NCORES):
        b, j = core // 4, core % 4
        q0 = j * CH
        kv_gk = np.full(KV, -1, np.int64)
        kv_gk[0:512] = q0 + np.arange(CH)
        halo = q0 - 256 + np.arange(256)
        kv_gk[512:768] = np.where(halo >= 0, halo, -1)
        kv_gk[768:772] = np.arange(4)

        xTc = np.zeros((C, KV), np.float32)
        valid = kv_gk >= 0
        xTc[:, valid] = x[b, kv_gk[valid]].T

        cq, sq = cos_sin(q0 + np.arange(CH))
        ck, sk = cos_sin(np.maximum(kv_gk, 0))

        gq = q0 + np.arange(CH)
        mask = np.zeros((128, MTOT), np.float32)
        for c in range(7):
            rows = CBASE[c] + np.arange(CROWS[c])
            gk = kv_gk[rows]
            qw = gq[OFF_C[c]:OFF_C[c] + W_C[c]]
            real = gk >= 0
            g = np.where(real, gk, 0)[:, None]
            qq = qw[None, :]
            if c == 6:
                allow = (g <= qq) & (qq - g >= WIN)
            else:
                allow = (g <= qq) & (qq - g < WIN)
            allow &= real[:, None]
            mask[0:CROWS[c], MOFF[c]:MOFF[c] + W_C[c]] = allow
        in_maps.append({
            "xT": np.ascontiguousarray(xTc.astype(BF)),
            "wqk": wqkm, "wv": wvm, "wps": wp,
            "cos_q": cq, "sin_q": sq, "cos_k": ck, "sin_k": sk,
            "masks": np.ascontiguousarray(mask.astype(BF)),
            "p2": P2,
        })
    return in_maps


def kernel(x, w_attn, w_proj):
    from concourse import bass_utils

    x = np.asarray(x, np.float32)
    w_attn = np.asarray(w_attn, np.float32)
    w_proj = np.asarray(w_proj, np.float32)

    if "nc" not in _cache:
        _cache["nc"] = _build_nc()
    nc = _cache["nc"]

    in_maps = _host_inputs(x, w_attn, w_proj)
    res = bass_utils.run_bass_kernel_spmd(nc, in_maps, list(range(NCORES)),
                                          **_cache.get("run_kwargs", {}))
    _cache["last_result"] = res

    y = np.zeros((B, T, C), np.float32)
    for core in range(NCORES):
        b, j = core // 4, core % 4
        y[b, j * CH:(j + 1) * CH, :] = res.results[core]["outT"].T
    return y


# revision 26
# speedup vs baseline: 1.1358x; 1.0432x over previous
"""Trainium2 Bass kernel for CausalSelfAttention (sliding window 256 + 4 sink).

v4: 8 cores = (batch 2) x (seq chunks of 512). Per core:
KV = 772 = [own 512 | halo 256 | sink 4], QKV + RoPE in bf16, banded
attention in scores-transposed [k, q] layout with 7 key chunks.

Changes over v3:
- PE warmup matmuls right after the first x tile lands (HAM K=8/8 by the
  time real work starts).
- AV col-split: both head-halves' AV chains run concurrently in the PE
  array (half0 -> psum partitions 0:64, half1 -> 64:128 via tile_position),
  yt is ONE [128, 512] psum tile, evacuated with ONE Act copy.
- Denominator+broadcast fused: ones[m, 64] stationary -> the PE both
  row-sums pt AND replicates the result across 64 partitions; the two
  halves' chains run concurrently. No dh copies, no sel broadcast MMs.
- reciprocal_approx_fast (1 DVE op, fp32) instead of iterative RECIPROCAL
  (3.4us -> ~0.7us, off the critical path).
- masks multiplied per group (in-place into praw) so AV starts earlier.
- V stored plain [128, 1024] (no ones column, no strided copies).
- proj psum evacuation on DVE (Act is the busier engine there).
"""

import numpy as np

B, T, C, NH, HD = 2, 2048, 1024, 16, 64
WIN, SINK = 256, 4
CH = 512          # queries per core
KV = 772          # 512 own + 256 halo + 4 sink
KVP = 784         # KV padded so fp8 pair-stride is 16B-aligned (DoubleRow ISA)
NCORES = 8
W_C = [384, 384, 256, 128, 128, 256, 512]
OFF_C = [0, 128, 256, 384, 0, 0, 0]
# Score chunks packed into psum banks of 512 cols each; praw/mask share
# this column layout. Sink group first: chunk 6 is the AV starter.
GROUPS = [[6], [0, 3], [1, 4], [2, 5]]
MOFF = np.zeros(8, int)
_off = 0
for _g in GROUPS:
    for _c in _g:
        MOFF[_c] = _off
        _off += W_C[_c]
MTOT = _off  # 2048
AVORD = [6, 0, 3, 1, 4, 2, 5]
CROWS = [128, 128, 128, 128, 128, 128, 4]
CBASE = [0, 128, 256, 384, 512, 640, 768]
KSEG = [(0, 512), (512, 260)]  # kv free-dim segments (psum bank limit)
NWARM = 14

_cache = {}


def _build_nc():
    import concourse.bacc as bacc
    import concourse.mybir as mybir
    import concourse.tile as tile

    f32 = mybir.dt.float32
    bf16 = mybir.dt.bfloat16
    fp8 = mybir.dt.float8e4
    DR = mybir.MatmulPerfMode.DoubleRow
    AF = mybir.ActivationFunctionType

    nc = bacc.Bacc("TRN2", target_bir_lowering=False, debug=False,
                   num_devices=NCORES)

    xT = nc.dram_tensor("xT", [C, KV], bf16, kind="ExternalInput").ap()
    wqk = nc.dram_tensor("wqk", [C, 2 * C], bf16, kind="ExternalInput").ap()
    wv = nc.dram_tensor("wv", [C, C], bf16, kind="ExternalInput").ap()
    wps = nc.dram_tensor("wps", [C, C], bf16, kind="ExternalInput").ap()
    cos_q = nc.dram_tensor("cos_q", [128, CH], bf16, kind="ExternalInput").ap()
    sin_q = nc.dram_tensor("sin_q", [128, CH], bf16, kind="ExternalInput").ap()
    cos_k = nc.dram_tensor("cos_k", [128, KV], bf16, kind="ExternalInput").ap()
    sin_k = nc.dram_tensor("sin_k", [128, KV], bf16, kind="ExternalInput").ap()
    masks = nc.dram_tensor("masks", [128, MTOT], bf16,
                           kind="ExternalInput").ap()
    p2d = nc.dram_tensor("p2", [128, 128], bf16, kind="ExternalInput").ap()
    outT = nc.dram_tensor("outT", [C, CH], f32, kind="ExternalOutput").ap()

    with tile.TileContext(nc) as tc:
        with (
            tc.tile_pool(name="pers", bufs=1) as pers,
            tc.tile_pool(name="qk", bufs=3) as qkp,
            tc.tile_pool(name="tmp", bufs=2) as tmp,
            tc.tile_pool(name="praw", bufs=2) as prp,
            tc.tile_pool(name="rcp", bufs=2) as rcpp,
            tc.tile_pool(name="osb", bufs=2) as osbp,
            tc.tile_pool(name="psmm", bufs=2, space="PSUM") as psmm,
            tc.tile_pool(name="pssc", bufs=3, space="PSUM") as pssc,
            tc.tile_pool(name="psyd", bufs=3, space="PSUM") as psyd,
        ):
            # ones block: denominator chains use cols 0:64; also the PE
            # warmup operand (ready ~immediately, unlike any DMA)
            tones = pers.tile([128, 512], bf16, tag="ones")
            nc.gpsimd.memset(tones[:], 1.0)

            # PE warmup: HAM un-throttles after ~3.4us of sustained PE
            # activity; burn the DMA-load window on dummy matmuls so real
            # matmuls run at 2.4 GHz from the start.
            for wi in range(NWARM):
                pw = psmm.tile([128, 512], f32, tag="mm", name=f"warm{wi}")
                nc.tensor.matmul(pw[:], tones[:, 0:128], tones[:],
                                 start=True, stop=True)

            # Batched loads: the Sync engine costs ~0.6us of issue time
            # per dma_start regardless of size, so the 8-way-split tensors
            # are loaded as ONE wide tile each and sliced with views.
            HB = KV - CH
            xqall = pers.tile([128, 8, CH], bf16, tag="xqall")
            for hf in range(2):
                nc.sync.dma_start(
                    xqall[:, hf * 4:(hf + 1) * 4, :],
                    xT[hf * 512:(hf + 1) * 512, 0:CH].rearrange(
                        "(i p) k -> p i k", p=128))
            xq = [xqall[:, i, :] for i in range(8)]

            wqkt = [None] * 8

            def load_wqk(i):
                t = pers.tile([128, 2048], bf16, tag=f"wqk{i}",
                              name=f"wqk{i}")
                nc.sync.dma_start(t[:], wqk[i * 128:(i + 1) * 128, :])
                wqkt[i] = t

            load_wqk(0)
            tcos_q = pers.tile([128, CH], bf16, tag="cos_q")
            nc.sync.dma_start(tcos_q[:], cos_q[:])
            tsin_q = pers.tile([128, CH], bf16, tag="sin_q")
            nc.sync.dma_start(tsin_q[:], sin_q[:])
            tp2 = pers.tile([128, 128], bf16, tag="p2")
            nc.sync.dma_start(tp2[:], p2d[:])
            xhall = pers.tile([128, 8, HB], bf16, tag="xhall")
            nc.sync.dma_start(
                xhall[:], xT[:, CH:KV].rearrange("(i p) k -> p i k", p=128))
            xh = [xhall[:, i, :] for i in range(8)]
            tcos_k = pers.tile([128, KV], bf16, tag="cos_k")
            nc.sync.dma_start(tcos_k[:], cos_k[:])
            tsin_k = pers.tile([128, KV], bf16, tag="sin_k")
            nc.sync.dma_start(tsin_k[:], sin_k[:])
            wvall = pers.tile([128, 8, 1024], bf16, tag="wvall")
            for hf in range(2):
                nc.sync.dma_start(
                    wvall[:, hf * 4:(hf + 1) * 4, :],
                    wv[hf * 512:(hf + 1) * 512, :].rearrange(
                        "(i p) n -> p i n", p=128))
            wvc = [wvall[:, i, :] for i in range(8)]
            load_wqk(1)
            tmask = pers.tile([128, MTOT], bf16, tag="mask")
            nc.sync.dma_start(tmask[:], masks[:])
            for i in range(2, 8):
                load_wqk(i)
            wpall = pers.tile([128, 8, 1024], bf16, tag="wpall")
            nc.sync.dma_start(
                wpall[:], wps.rearrange("(i p) n -> p i n", p=128))
            wpsl = [wpall[:, i, :] for i in range(8)]

            def qkv_rope(hp):
                # qT raw -> rope
                pq = psmm.tile([128, 512], f32, tag="mm", name=f"pq{hp}")
                for kc in range(8):
                    nc.tensor.matmul(
                        pq[:], wqkt[hp][:, kc * 128:(kc + 1) * 128],
                        xq[kc][:],
                        start=(kc == 0), stop=(kc == 7),
                    )
                qraw = tmp.tile([128, CH], bf16, tag="qraw", name=f"qraw{hp}")
                nc.scalar.copy(qraw[:], pq[:])
                qT = qkp.tile([128, CH], bf16, tag="qT", name=f"qT{hp}")
                prot = psmm.tile([128, 512], f32, tag="mm", name=f"prot{hp}")
                nc.tensor.matmul(prot[:], tp2[:], qraw[:], start=True,
                                 stop=True)
                t2 = tmp.tile([128, CH], bf16, tag="t2", name=f"t2q{hp}")
                nc.vector.tensor_mul(t2[:], prot[:], tsin_q[:])
                qm = tmp.tile([128, CH], bf16, tag="qm", name=f"qm{hp}")
                nc.vector.tensor_mul(qm[:], qraw[:], tcos_q[:])
                nc.vector.tensor_add(qT[:], qm[:], t2[:])

                # kT raw -> rope (two psum segments)
                kraw = tmp.tile([128, KV], bf16, tag="kraw", name=f"kraw{hp}")
                pk = [psmm.tile([128, 512], f32, tag="mm", name=f"pk{hp}_{i}")
                      for i in range(2)]
                for kc in range(8):
                    for si, (s0, sw) in enumerate(KSEG):
                        rhs = xq[kc][:] if si == 0 else xh[kc][:, 0:sw]
                        nc.tensor.matmul(
                            pk[si][:, 0:sw],
                            wqkt[hp][:, 1024 + kc * 128:1024 + (kc + 1) * 128],
                            rhs,
                            start=(kc == 0), stop=(kc == 7),
                        )
                kT = qkp.tile([128, KV], bf16, tag="kT", name=f"kT{hp}")
                for si, (s0, sw) in enumerate(KSEG):
                    nc.scalar.copy(kraw[:, s0:s0 + sw], pk[si][:, 0:sw])
                    prk = psmm.tile([128, 512], f32, tag="mm",
                                    name=f"prk{hp}_{si}")
                    nc.tensor.matmul(prk[:, 0:sw], tp2[:],
                                     kraw[:, s0:s0 + sw], start=True,
                                     stop=True)
                    t2k = tmp.tile([128, 512], bf16, tag="t2",
                                   name=f"t2k{hp}_{si}")
                    nc.vector.tensor_mul(t2k[:, 0:sw], prk[:, 0:sw],
                                         tsin_k[:, s0:s0 + sw])
                    km = tmp.tile([128, 512], bf16, tag="qm",
                                  name=f"km{hp}_{si}")
                    nc.vector.tensor_mul(km[:, 0:sw], kraw[:, s0:s0 + sw],
                                         tcos_k[:, s0:s0 + sw])
                    nc.vector.tensor_add(kT[:, s0:s0 + sw], km[:, 0:sw],
                                         t2k[:, 0:sw])
                return qT, kT

            def sc_exp_block(hp, qT, kT):
                # scoresT: per chunk, two head-halves at partition bases 0/64
                # run concurrently in the PE array. Chunks within a GROUP
                # share one 512-wide psum bank, so each (group, half) needs
                # only ONE exp (fused 1/8 scale) into the contiguous
                # [128, MTOT] praw tile per half.
                praws = []
                for half in range(2):
                    praws.append(prp.tile([128, MTOT], bf16, tag="praw",
                                          name=f"praw{hp}_{half}"))
                for gi, grp in enumerate(GROUPS):
                    g0 = MOFF[grp[0]]
                    gw = sum(W_C[c] for c in grp)
                    gm = max(CROWS[c] for c in grp)
                    for half in range(2):
                        dsl = slice(half * 64, half * 64 + 64)
                        sc = pssc.tile([128, 512], f32, tag="sc",
                                       name=f"sc{hp}_{gi}_{half}")
                        for c in grp:
                            w, off, m = W_C[c], OFF_C[c], CROWS[c]
                            base = CBASE[c]
                            co = MOFF[c] - g0
                            nc.tensor.matmul(
                                sc[0:m, co:co + w], kT[dsl, base:base + m],
                                qT[dsl, off:off + w], start=True, stop=True,
                            )
                        nc.scalar.activation(
                            praws[half][0:gm, g0:g0 + gw],
                            sc[0:gm, 0:gw], AF.Exp, scale=0.125)
                return praws

            def mask_block(hp, praws):
                # multiplicative 0/1 mask, in place, two muls per half
                # aligned with the AV chunk order: cols 0:1024 covers
                # groups g0+g1 (AV chunks 6,0,3) so their AV matmuls gate
                # only on the first two exps; cols 1024:2048 covers g2+g3.
                for half in range(2):
                    nc.vector.tensor_mul(
                        praws[half][:, 0:1024], praws[half][:, 0:1024],
                        tmask[:, 0:1024])
                    nc.vector.tensor_mul(
                        praws[half][:, 1024:MTOT], praws[half][:, 1024:MTOT],
                        tmask[:, 1024:MTOT])
                return praws

            def av_alloc(hp):
                # yt/dd banks are DVE-memset to zero well before the AV
                # chains need them (the memset must not sit on the AV
                # critical path); all AV/dd matmuls then use start=False,
                # which is order-safe: DVE writes don't touch has_written,
                # so each first matmul either accumulates onto 0 or
                # overwrites.
                yt = psyd.tile([128, 512], f32, tag="yd", name=f"yt{hp}")
                dd = psyd.tile([128, 512], f32, tag="yd", name=f"dd{hp}")
                nc.vector.memset(yt[:], 0.0)
                nc.vector.memset(dd[:], 0.0)
                return yt, dd

            def av_norm_block(hp, pts, yt, dd):
                # One yt bank + one dd bank per hp: half0's outputs at psum
                # partitions 0:64, half1's at 64:128 (tile_position col
                # group from out base partition), so the two halves' chains
                # run concurrently in the PE array. dd is the fused
                # denominator+broadcast: stationary ones[m, 64] row-sums
                # pt AND replicates across partitions.
                h = [hp * 2, hp * 2 + 1]
                for ci, c in enumerate(AVORD):
                    w, off, m = W_C[c], OFF_C[c], CROWS[c]
                    for half in range(2):
                        nc.tensor.matmul(
                            yt[half * 64:half * 64 + 64, off:off + w],
                            v_sb[c][0:m, h[half] * 64:(h[half] + 1) * 64],
                            pts[half][0:m, MOFF[c]:MOFF[c] + w],
                            start=False, stop=(ci == 6 and half == 1),
                            skip_group_check=True,
                        )
                for ci, c in enumerate(AVORD):
                    w, off, m = W_C[c], OFF_C[c], CROWS[c]
                    for half in range(2):
                        nc.tensor.matmul(
                            dd[half * 64:half * 64 + 64, off:off + w],
                            tones[0:m, 0:64],
                            pts[half][0:m, MOFF[c]:MOFF[c] + w],
                            start=False, stop=(ci == 6 and half == 1),
                            skip_group_check=True,
                        )
                # reciprocal_approx_fast needs SBUF f32 source (PSUM source
                # returns garbage — HW-probed), so evacuate dd via Act first
                dsb = rcpp.tile([128, 512], f32, tag="dsb", name=f"dsb{hp}")
                rcp = rcpp.tile([128, 512], f32, tag="rcp", name=f"rcp{hp}")
                ytu = pers.tile([128, CH], bf16, tag=f"ytu{hp}",
                                name=f"ytu{hp}")
                nc.vector.tensor_copy(dsb[:], dd[:])
                nc.vector.reciprocal_approx_fast(rcp[:], dsb[:])
                nc.vector.tensor_mul(ytu[:], yt[:], rcp[:])
                return ytu

            # ---------- prologue: rope(0), V, rope(1) ----------
            qk_state = [None] * 8
            qk_state[0] = qkv_rope(0)

            v_sb = []
            for tt in range(7):
                m = CROWS[tt]
                vt = pers.tile([128, 1024], bf16, tag=f"v{tt}", name=f"v{tt}")
                pv = [psmm.tile([128, 512], f32, tag="mm", name=f"pv{tt}_{i}")
                      for i in range(2)]
                for kc in range(8):
                    if CBASE[tt] < CH:
                        xsl = xq[kc][:, CBASE[tt]:CBASE[tt] + m]
                    else:
                        xsl = xh[kc][:, CBASE[tt] - CH:CBASE[tt] - CH + m]
                    for dh in range(2):
                        nc.tensor.matmul(
                            pv[dh][0:m, :], xsl,
                            wvc[kc][:, dh * 512:(dh + 1) * 512],
                            start=(kc == 0), stop=(kc == 7),
                        )
                for dh in range(2):
                    nc.scalar.copy(
                        vt[0:m, dh * 512:(dh + 1) * 512], pv[dh][0:m, :])
                v_sb.append(vt)

            qk_state[1] = qkv_rope(1)

            # ---------- software-pipelined head-pair loop (rope depth 2).
            # The first-half projection chains (over hp 0..3) are emitted
            # inside the loop once yts[0..3] exist: they fill PE idle in
            # the rope-less tail iterations and keep HAM at K=8/8.
            yts = []
            acc_a = [None] * 8
            for hp in range(8):
                ytdd = av_alloc(hp)
                praws = sc_exp_block(hp, *qk_state[hp])
                pts = mask_block(hp, praws)
                if hp + 2 < 8:
                    qk_state[hp + 2] = qkv_rope(hp + 2)
                yts.append(av_norm_block(hp, pts, *ytdd))
                # rope is fully emitted by hp==5, so psmm is free of rope
                # tiles from here on: fill the rope-less tail iterations
                # with the first-half projection chains (hp 0..3).
                if 5 <= hp <= 7:
                    for cc in (2 * (hp - 5), 2 * (hp - 5) + 1):
                        po = psmm.tile([128, 512], f32, tag="mm",
                                       name=f"poa{cc}")
                        for hpp in range(4):
                            nc.tensor.matmul(
                                po[:], wpsl[cc][:, hpp * 128:(hpp + 1) * 128],
                                yts[hpp][:],
                                start=(hpp == 0), stop=(hpp == 3),
                            )
                        a = pers.tile([128, 512], f32, tag=f"acc{cc}",
                                      name=f"acc{cc}")
                        nc.vector.tensor_copy(a[:], po[:])
                        acc_a[cc] = a

            # ---------- projection second half + combine (transposed) ----
            for cc in range(8):
                po = psmm.tile([128, 512], f32, tag="mm", name=f"po{cc}")
                hp0 = 4 if cc < 6 else 0
                for hp in range(hp0, 8):
                    nc.tensor.matmul(
                        po[:], wpsl[cc][:, hp * 128:(hp + 1) * 128],
                        yts[hp][:],
                        start=(hp == hp0), stop=(hp == 7),
                    )
                osb = osbp.tile([128, 512], f32, tag="osb", name=f"osb{cc}")
                if cc < 6:
                    nc.vector.tensor_add(osb[:], po[:], acc_a[cc][:])
                else:
                    nc.vector.tensor_copy(osb[:], po[:])
                nc.sync.dma_start(outT[cc * 128:(cc + 1) * 128, :], osb[:])

    nc.compile()
    return nc


def _host_inputs(x, w_attn, w_proj):
    """Build the 8 per-core input maps (bf16 + fp8 pair layouts)."""
    import ml_dtypes
    BF = ml_dtypes.bfloat16
    E4 = ml_dtypes.float8_e4m3

    inv_freq = 1.0 / (10000.0 ** (np.arange(0, HD, 2, dtype=np.float32) / HD))
    iff = np.concatenate([inv_freq, inv_freq])  # [64]

    def cos_sin(pos):
        ang = pos[None, :].astype(np.float32) * iff[:, None]
        c = np.concatenate([np.cos(ang), np.cos(ang)], 0)
        s = np.concatenate([np.sin(ang), np.sin(ang)], 0)
        return np.ascontiguousarray(c.astype(BF)), \
            np.ascontiguousarray(s.astype(BF))

    P2 = np.zeros((128, 128), np.float32)
    for blk in range(2):
        o = blk * 64
        for d in range(32):
            P2[o + d + 32, o + d] = -1.0
            P2[o + d, o + d + 32] = 1.0
    P2 = P2.astype(BF)

    def shuffle_lhsT(w):
        # rows (kc*128 + c_lo), cols (hp*128 + d) ->
        # rows (hp*128 + c_lo), cols (kc*128 + d)
        return np.ascontiguousarray(
            w.reshape(8, 128, 8, 128).transpose(2, 1, 0, 3).reshape(C, C)
        )

    wq = shuffle_lhsT(w_attn[:, 0:C]).astype(BF)
    wk = shuffle_lhsT(w_attn[:, C:2 * C]).astype(BF)
    wqkm = np.ascontiguousarray(np.concatenate([wq, wk], axis=1))
    wvm = np.ascontiguousarray(w_attn[:, 2 * C:3 * C]).astype(BF)
    wp = shuffle_lhsT(w_proj).astype(BF)

    in_maps = []
    for core in range(NCORES):
        b, j = core // 4, core % 4
        q0 = j * CH
        kv_gk = np.full(KV, -1, np.int64)
        kv_gk[0:512] = q0 + np.arange(CH)
        halo = q0 - 256 + np.arange(256)
        kv_gk[512:768] = np.where(halo >= 0, halo, -1)
        kv_gk[768:772] = np.arange(4)

        xTc = np.zeros((C, KV), np.float32)
        valid = kv_gk >= 0
        xTc[:, valid] = x[b, kv_gk[valid]].T

        cq, sq = cos_sin(q0 + np.arange(CH))
        ck, sk = cos_sin(np.maximum(kv_gk, 0))

        gq = q0 + np.arange(CH)
        mask = np.zeros((128, MTOT), np.float32)
        for c in range(7):
            rows = CBASE[c] + np.arange(CROWS[c])
            gk = kv_gk[rows]
            qw = gq[OFF_C[c]:OFF_C[c] + W_C[c]]
            real = gk >= 0
            g = np.where(real, gk, 0)[:, None]
            qq = qw[None, :]
            if c == 6:
                allow = (g <= qq) & (qq - g >= WIN)
            else:
                allow = (g <= qq) & (qq - g < WIN)
            allow &= real[:, None]
            mask[0:CROWS[c], MOFF[c]:MOFF[c] + W_C[c]] = allow
        in_maps.append({
            "xT": np.ascontiguousarray(xTc.astype(BF)),
            "wqk": wqkm, "wv": wvm, "wps": wp,
            "cos_q": cq, "sin_q": sq, "cos_k": ck, "sin_k": sk,
            "masks": np.ascontiguousarray(mask.astype(BF)),
            "p2": P2,
        })
    return in_maps


def kernel(x, w_attn, w_proj):
    from concourse import bass_utils

    x = np.asarray(x, np.float32)
    w_attn = np.asarray(w_attn, np.float32)
    w_proj = np.asarray(w_proj, np.float32)

    if "nc" not in _cache:
        _cache["nc"] = _build_nc()
    nc = _cache["nc"]

    in_maps = _host_inputs(x, w_attn, w_proj)
    res = bass_utils.run_bass_kernel_spmd(nc, in_maps, list(range(NCORES)),
                                          **_cache.get("run_kwargs", {}))
    _cache["last_result"] = res

    y = np.zeros((B, T, C), np.float32)
    for core in range(NCORES):
        b, j = core // 4, core % 4
        y[b, j * CH:(j + 1) * CH, :] = res.results[core]["outT"].T
    return y


# revision 27
# speedup vs baseline: 1.1617x; 1.0228x over previous
"""Trainium2 Bass kernel for CausalSelfAttention (sliding window 256 + 4 sink).

v4: 8 cores = (batch 2) x (seq chunks of 512). Per core:
KV = 772 = [own 512 | halo 256 | sink 4], QKV + RoPE in bf16, banded
attention in scores-transposed [k, q] layout with 7 key chunks.

Changes over v3:
- PE warmup matmuls right after the first x tile lands (HAM K=8/8 by the
  time real work starts).
- AV col-split: both head-halves' AV chains run concurrently in the PE
  array (half0 -> psum partitions 0:64, half1 -> 64:128 via tile_position),
  yt is ONE [128, 512] psum tile, evacuated with ONE Act copy.
- Denominator+broadcast fused: ones[m, 64] stationary -> the PE both
  row-sums pt AND replicates the result across 64 partitions; the two
  halves' chains run concurrently. No dh copies, no sel broadcast MMs.
- reciprocal_approx_fast (1 DVE op, fp32) instead of iterative RECIPROCAL
  (3.4us -> ~0.7us, off the critical path).
- masks multiplied per group (in-place into praw) so AV starts earlier.
- V stored plain [128, 1024] (no ones column, no strided copies).
- proj psum evacuation on DVE (Act is the busier engine there).
"""

import numpy as np

B, T, C, NH, HD = 2, 2048, 1024, 16, 64
WIN, SINK = 256, 4
CH = 512          # queries per core
KV = 772          # 512 own + 256 halo + 4 sink
KVP = 784         # KV padded so fp8 pair-stride is 16B-aligned (DoubleRow ISA)
NCORES = 8
W_C = [384, 384, 256, 128, 128, 256, 512]
OFF_C = [0, 128, 256, 384, 0, 0, 0]
# Score chunks packed into psum banks of 512 cols each; praw/mask share
# this column layout. Sink group first: chunk 6 is the AV starter.
GROUPS = [[6], [0, 3], [1, 4], [2, 5]]
MOFF = np.zeros(8, int)
_off = 0
for _g in GROUPS:
    for _c in _g:
        MOFF[_c] = _off
        _off += W_C[_c]
MTOT = _off  # 2048
AVORD = [6, 0, 3, 1, 4, 2, 5]
CROWS = [128, 128, 128, 128, 128, 128, 4]
CBASE = [0, 128, 256, 384, 512, 640, 768]
KSEG = [(0, 512), (512, 260)]  # kv free-dim segments (psum bank limit)
NWARM = 16

_cache = {}


def _build_nc():
    import concourse.bacc as bacc
    import concourse.mybir as mybir
    import concourse.tile as tile

    f32 = mybir.dt.float32
    bf16 = mybir.dt.bfloat16
    fp8 = mybir.dt.float8e4
    DR = mybir.MatmulPerfMode.DoubleRow
    AF = mybir.ActivationFunctionType

    nc = bacc.Bacc("TRN2", target_bir_lowering=False, debug=False,
                   num_devices=NCORES)

    xT = nc.dram_tensor("xT", [C, KV], bf16, kind="ExternalInput").ap()
    wqk = nc.dram_tensor("wqk", [C, 2 * C], bf16, kind="ExternalInput").ap()
    wv = nc.dram_tensor("wv", [C, C], bf16, kind="ExternalInput").ap()
    wps = nc.dram_tensor("wps", [C, C], bf16, kind="ExternalInput").ap()
    cos_q = nc.dram_tensor("cos_q", [128, CH], bf16, kind="ExternalInput").ap()
    sin_q = nc.dram_tensor("sin_q", [128, CH], bf16, kind="ExternalInput").ap()
    cos_k = nc.dram_tensor("cos_k", [128, KV], bf16, kind="ExternalInput").ap()
    sin_k = nc.dram_tensor("sin_k", [128, KV], bf16, kind="ExternalInput").ap()
    masks = nc.dram_tensor("masks", [128, MTOT], bf16,
                           kind="ExternalInput").ap()
    p2d = nc.dram_tensor("p2", [128, 128], bf16, kind="ExternalInput").ap()
    outT = nc.dram_tensor("outT", [C, CH], f32, kind="ExternalOutput").ap()

    with tile.TileContext(nc) as tc:
        with (
            tc.tile_pool(name="pers", bufs=1) as pers,
            tc.tile_pool(name="qk", bufs=3) as qkp,
            tc.tile_pool(name="tmp", bufs=2) as tmp,
            tc.tile_pool(name="praw", bufs=2) as prp,
            tc.tile_pool(name="rcp", bufs=2) as rcpp,
            tc.tile_pool(name="osb", bufs=3) as osbp,
            tc.tile_pool(name="psmm", bufs=2, space="PSUM") as psmm,
            tc.tile_pool(name="pssc", bufs=3, space="PSUM") as pssc,
            tc.tile_pool(name="psyd", bufs=3, space="PSUM") as psyd,
        ):
            # ones block: denominator chains use cols 0:64; also the PE
            # warmup operand (ready ~immediately, unlike any DMA)
            tones = pers.tile([128, 512], bf16, tag="ones")
            nc.gpsimd.memset(tones[:], 1.0)

            # PE warmup: HAM un-throttles after ~3.4us of sustained PE
            # activity; burn the DMA-load window on dummy matmuls so real
            # matmuls run at 2.4 GHz from the start.
            for wi in range(NWARM):
                pw = psmm.tile([128, 512], f32, tag="mm", name=f"warm{wi}")
                nc.tensor.matmul(pw[:], tones[:, 0:128], tones[:],
                                 start=True, stop=True)

            # Batched loads: the Sync engine costs ~0.6us of issue time
            # per dma_start regardless of size, so the 8-way-split tensors
            # are loaded as ONE wide tile each and sliced with views.
            HB = KV - CH
            xqall = pers.tile([128, 8, CH], bf16, tag="xqall")
            for hf in range(2):
                nc.sync.dma_start(
                    xqall[:, hf * 4:(hf + 1) * 4, :],
                    xT[hf * 512:(hf + 1) * 512, 0:CH].rearrange(
                        "(i p) k -> p i k", p=128))
            xq = [xqall[:, i, :] for i in range(8)]

            wqkt = [None] * 8

            def load_wqk(i):
                t = pers.tile([128, 2048], bf16, tag=f"wqk{i}",
                              name=f"wqk{i}")
                nc.sync.dma_start(t[:], wqk[i * 128:(i + 1) * 128, :])
                wqkt[i] = t

            load_wqk(0)
            tcos_q = pers.tile([128, CH], bf16, tag="cos_q")
            nc.sync.dma_start(tcos_q[:], cos_q[:])
            tsin_q = pers.tile([128, CH], bf16, tag="sin_q")
            nc.sync.dma_start(tsin_q[:], sin_q[:])
            tp2 = pers.tile([128, 128], bf16, tag="p2")
            nc.sync.dma_start(tp2[:], p2d[:])
            xhall = pers.tile([128, 8, HB], bf16, tag="xhall")
            nc.sync.dma_start(
                xhall[:], xT[:, CH:KV].rearrange("(i p) k -> p i k", p=128))
            xh = [xhall[:, i, :] for i in range(8)]
            tcos_k = pers.tile([128, KV], bf16, tag="cos_k")
            nc.sync.dma_start(tcos_k[:], cos_k[:])
            tsin_k = pers.tile([128, KV], bf16, tag="sin_k")
            nc.sync.dma_start(tsin_k[:], sin_k[:])
            wvall = pers.tile([128, 8, 1024], bf16, tag="wvall")
            for hf in range(2):
                nc.sync.dma_start(
                    wvall[:, hf * 4:(hf + 1) * 4, :],
                    wv[hf * 512:(hf + 1) * 512, :].rearrange(
                        "(i p) n -> p i n", p=128))
            wvc = [wvall[:, i, :] for i in range(8)]
            load_wqk(1)
            tmask = pers.tile([128, MTOT], bf16, tag="mask")
            nc.sync.dma_start(tmask[:], masks[:])
            for i in range(2, 8):
                load_wqk(i)
            wpall = pers.tile([128, 8, 1024], bf16, tag="wpall")
            nc.sync.dma_start(
                wpall[:], wps.rearrange("(i p) n -> p i n", p=128))
            wpsl = [wpall[:, i, :] for i in range(8)]

            def qkv_rope(hp):
                # qT raw -> rope
                pq = psmm.tile([128, 512], f32, tag="mm", name=f"pq{hp}")
                for kc in range(8):
                    nc.tensor.matmul(
                        pq[:], wqkt[hp][:, kc * 128:(kc + 1) * 128],
                        xq[kc][:],
                        start=(kc == 0), stop=(kc == 7),
                    )
                qraw = tmp.tile([128, CH], bf16, tag="qraw", name=f"qraw{hp}")
                nc.scalar.copy(qraw[:], pq[:])
                qT = qkp.tile([128, CH], bf16, tag="qT", name=f"qT{hp}")
                prot = psmm.tile([128, 512], f32, tag="mm", name=f"prot{hp}")
                nc.tensor.matmul(prot[:], tp2[:], qraw[:], start=True,
                                 stop=True)
                t2 = tmp.tile([128, CH], bf16, tag="t2", name=f"t2q{hp}")
                nc.vector.tensor_mul(t2[:], prot[:], tsin_q[:])
                qm = tmp.tile([128, CH], bf16, tag="qm", name=f"qm{hp}")
                nc.vector.tensor_mul(qm[:], qraw[:], tcos_q[:])
                nc.vector.tensor_add(qT[:], qm[:], t2[:])

                # kT raw -> rope (two psum segments)
                kraw = tmp.tile([128, KV], bf16, tag="kraw", name=f"kraw{hp}")
                pk = [psmm.tile([128, 512], f32, tag="mm", name=f"pk{hp}_{i}")
                      for i in range(2)]
                for kc in range(8):
                    for si, (s0, sw) in enumerate(KSEG):
                        rhs = xq[kc][:] if si == 0 else xh[kc][:, 0:sw]
                        nc.tensor.matmul(
                            pk[si][:, 0:sw],
                            wqkt[hp][:, 1024 + kc * 128:1024 + (kc + 1) * 128],
                            rhs,
                            start=(kc == 0), stop=(kc == 7),
                        )
                kT = qkp.tile([128, KV], bf16, tag="kT", name=f"kT{hp}")
                for si, (s0, sw) in enumerate(KSEG):
                    nc.scalar.copy(kraw[:, s0:s0 + sw], pk[si][:, 0:sw])
                    prk = psmm.tile([128, 512], f32, tag="mm",
                                    name=f"prk{hp}_{si}")
                    nc.tensor.matmul(prk[:, 0:sw], tp2[:],
                                     kraw[:, s0:s0 + sw], start=True,
                                     stop=True)
                    t2k = tmp.tile([128, 512], bf16, tag="t2",
                                   name=f"t2k{hp}_{si}")
                    nc.vector.tensor_mul(t2k[:, 0:sw], prk[:, 0:sw],
                                         tsin_k[:, s0:s0 + sw])
                    km = tmp.tile([128, 512], bf16, tag="qm",
                                  name=f"km{hp}_{si}")
                    nc.vector.tensor_mul(km[:, 0:sw], kraw[:, s0:s0 + sw],
                                         tcos_k[:, s0:s0 + sw])
                    nc.vector.tensor_add(kT[:, s0:s0 + sw], km[:, 0:sw],
                                         t2k[:, 0:sw])
                return qT, kT

            def sc_exp_block(hp, qT, kT):
                # scoresT: per chunk, two head-halves at partition bases 0/64
                # run concurrently in the PE array. Chunks within a GROUP
                # share one 512-wide psum bank, so each (group, half) needs
                # only ONE exp (fused 1/8 scale) into the contiguous
                # [128, MTOT] praw tile per half.
                praws = []
                for half in range(2):
                    praws.append(prp.tile([128, MTOT], bf16, tag="praw",
                                          name=f"praw{hp}_{half}"))
                for gi, grp in enumerate(GROUPS):
                    g0 = MOFF[grp[0]]
                    gw = sum(W_C[c] for c in grp)
                    gm = max(CROWS[c] for c in grp)
                    for half in range(2):
                        dsl = slice(half * 64, half * 64 + 64)
                        sc = pssc.tile([128, 512], f32, tag="sc",
                                       name=f"sc{hp}_{gi}_{half}")
                        for c in grp:
                            w, off, m = W_C[c], OFF_C[c], CROWS[c]
                            base = CBASE[c]
                            co = MOFF[c] - g0
                            nc.tensor.matmul(
                                sc[0:m, co:co + w], kT[dsl, base:base + m],
                                qT[dsl, off:off + w], start=True, stop=True,
                            )
                        nc.scalar.activation(
                            praws[half][0:gm, g0:g0 + gw],
                            sc[0:gm, 0:gw], AF.Exp, scale=0.125)
                return praws

            def mask_block(hp, praws):
                # multiplicative 0/1 mask, in place, two muls per half
                # aligned with the AV chunk order: cols 0:1024 covers
                # groups g0+g1 (AV chunks 6,0,3) so their AV matmuls gate
                # only on the first two exps; cols 1024:2048 covers g2+g3.
                for half in range(2):
                    nc.vector.tensor_mul(
                        praws[half][:, 0:1024], praws[half][:, 0:1024],
                        tmask[:, 0:1024])
                    nc.vector.tensor_mul(
                        praws[half][:, 1024:MTOT], praws[half][:, 1024:MTOT],
                        tmask[:, 1024:MTOT])
                return praws

            def av_alloc(hp):
                # yt/dd banks are DVE-memset to zero well before the AV
                # chains need them (the memset must not sit on the AV
                # critical path); all AV/dd matmuls then use start=False,
                # which is order-safe: DVE writes don't touch has_written,
                # so each first matmul either accumulates onto 0 or
                # overwrites.
                yt = psyd.tile([128, 512], f32, tag="yd", name=f"yt{hp}")
                dd = psyd.tile([128, 512], f32, tag="yd", name=f"dd{hp}")
                nc.vector.memset(yt[:], 0.0)
                nc.vector.memset(dd[:], 0.0)
                return yt, dd

            def av_norm_block(hp, pts, yt, dd):
                # One yt bank + one dd bank per hp: half0's outputs at psum
                # partitions 0:64, half1's at 64:128 (tile_position col
                # group from out base partition), so the two halves' chains
                # run concurrently in the PE array. dd is the fused
                # denominator+broadcast: stationary ones[m, 64] row-sums
                # pt AND replicates across partitions.
                h = [hp * 2, hp * 2 + 1]
                for ci, c in enumerate(AVORD):
                    w, off, m = W_C[c], OFF_C[c], CROWS[c]
                    for half in range(2):
                        nc.tensor.matmul(
                            yt[half * 64:half * 64 + 64, off:off + w],
                            v_sb[c][0:m, h[half] * 64:(h[half] + 1) * 64],
                            pts[half][0:m, MOFF[c]:MOFF[c] + w],
                            start=False, stop=(ci == 6 and half == 1),
                            skip_group_check=True,
                        )
                for ci, c in enumerate(AVORD):
                    w, off, m = W_C[c], OFF_C[c], CROWS[c]
                    for half in range(2):
                        nc.tensor.matmul(
                            dd[half * 64:half * 64 + 64, off:off + w],
                            tones[0:m, 0:64],
                            pts[half][0:m, MOFF[c]:MOFF[c] + w],
                            start=False, stop=(ci == 6 and half == 1),
                            skip_group_check=True,
                        )
                # reciprocal_approx_fast needs SBUF f32 source (PSUM source
                # returns garbage — HW-probed), so evacuate dd via Act first
                dsb = rcpp.tile([128, 512], f32, tag="dsb", name=f"dsb{hp}")
                rcp = rcpp.tile([128, 512], f32, tag="rcp", name=f"rcp{hp}")
                ytu = pers.tile([128, CH], bf16, tag=f"ytu{hp}",
                                name=f"ytu{hp}")
                nc.vector.tensor_copy(dsb[:], dd[:])
                nc.vector.reciprocal_approx_fast(rcp[:], dsb[:])
                nc.vector.tensor_mul(ytu[:], yt[:], rcp[:])
                return ytu

            # ---------- prologue: rope(0), V, rope(1) ----------
            qk_state = [None] * 8
            qk_state[0] = qkv_rope(0)

            v_sb = []
            for tt in range(7):
                m = CROWS[tt]
                vt = pers.tile([128, 1024], bf16, tag=f"v{tt}", name=f"v{tt}")
                pv = [psmm.tile([128, 512], f32, tag="mm", name=f"pv{tt}_{i}")
                      for i in range(2)]
                for kc in range(8):
                    if CBASE[tt] < CH:
                        xsl = xq[kc][:, CBASE[tt]:CBASE[tt] + m]
                    else:
                        xsl = xh[kc][:, CBASE[tt] - CH:CBASE[tt] - CH + m]
                    for dh in range(2):
                        nc.tensor.matmul(
                            pv[dh][0:m, :], xsl,
                            wvc[kc][:, dh * 512:(dh + 1) * 512],
                            start=(kc == 0), stop=(kc == 7),
                        )
                for dh in range(2):
                    nc.scalar.copy(
                        vt[0:m, dh * 512:(dh + 1) * 512], pv[dh][0:m, :])
                v_sb.append(vt)

            qk_state[1] = qkv_rope(1)

            # ---------- software-pipelined head-pair loop (rope depth 2).
            # The first-half projection chains (over hp 0..3) are emitted
            # inside the loop once yts[0..3] exist: they fill PE idle in
            # the rope-less tail iterations and keep HAM at K=8/8.
            yts = []
            acc_a = [None] * 8
            for hp in range(8):
                ytdd = av_alloc(hp)
                praws = sc_exp_block(hp, *qk_state[hp])
                pts = mask_block(hp, praws)
                if hp + 2 < 8:
                    qk_state[hp + 2] = qkv_rope(hp + 2)
                yts.append(av_norm_block(hp, pts, *ytdd))
                # rope is fully emitted by hp==5, so psmm is free of rope
                # tiles from here on: fill the rope-less tail iterations
                # with the first-half projection chains (hp 0..3).
                if 5 <= hp <= 7:
                    for cc in (2 * (hp - 5), 2 * (hp - 5) + 1):
                        po = psmm.tile([128, 512], f32, tag="mm",
                                       name=f"poa{cc}")
                        for hpp in range(4):
                            nc.tensor.matmul(
                                po[:], wpsl[cc][:, hpp * 128:(hpp + 1) * 128],
                                yts[hpp][:],
                                start=(hpp == 0), stop=(hpp == 3),
                            )
                        a = pers.tile([128, 512], f32, tag=f"acc{cc}",
                                      name=f"acc{cc}")
                        nc.scalar.copy(a[:], po[:])
                        acc_a[cc] = a

            # ---------- projection second half + combine (transposed) ----
            for cc in range(8):
                po = psmm.tile([128, 512], f32, tag="mm", name=f"po{cc}")
                hp0 = 4 if cc < 6 else 0
                for hp in range(hp0, 8):
                    nc.tensor.matmul(
                        po[:], wpsl[cc][:, hp * 128:(hp + 1) * 128],
                        yts[hp][:],
                        start=(hp == hp0), stop=(hp == 7),
                    )
                osb = osbp.tile([128, 512], f32, tag="osb", name=f"osb{cc}")
                if cc < 6:
                    nc.vector.tensor_add(osb[:], po[:], acc_a[cc][:])
                else:
                    nc.vector.tensor_copy(osb[:], po[:])
                nc.sync.dma_start(outT[cc * 128:(cc + 1) * 128, :], osb[:])

    nc.compile()
    return nc


def _host_inputs(x, w_attn, w_proj):
    """Build the 8 per-core input maps (bf16 + fp8 pair layouts)."""
    import ml_dtypes
    BF = ml_dtypes.bfloat16
    E4 = ml_dtypes.float8_e4m3

    inv_freq = 1.0 / (10000.0 ** (np.arange(0, HD, 2, dtype=np.float32) / HD))
    iff = np.concatenate([inv_freq, inv_freq])  # [64]

    def cos_sin(pos):
        ang = pos[None, :].astype(np.float32) * iff[:, None]
        c = np.concatenate([np.cos(ang), np.cos(ang)], 0)
        s = np.concatenate([np.sin(ang), np.sin(ang)], 0)
        return np.ascontiguousarray(c.astype(BF)), \
            np.ascontiguousarray(s.astype(BF))

    P2 = np.zeros((128, 128), np.float32)
    for blk in range(2):
        o = blk * 64
        for d in range(32):
            P2[o + d + 32, o + d] = -1.0
            P2[o + d, o + d + 32] = 1.0
    P2 = P2.astype(BF)

    def shuffle_lhsT(w):
        # rows (kc*128 + c_lo), cols (hp*128 + d) ->
        # rows (hp*128 + c_lo), cols (kc*128 + d)
        return np.ascontiguousarray(
            w.reshape(8, 128, 8, 128).transpose(2, 1, 0, 3).reshape(C, C)
        )

    wq = shuffle_lhsT(w_attn[:, 0:C]).astype(BF)
    wk = shuffle_lhsT(w_attn[:, C:2 * C]).astype(BF)
    wqkm = np.ascontiguousarray(np.concatenate([wq, wk], axis=1))
    wvm = np.ascontiguousarray(w_attn[:, 2 * C:3 * C]).astype(BF)
    wp = shuffle_lhsT(w_proj).astype(BF)

    in_maps = []
    for core in range(NCORES):
        b, j = core // 4, core % 4
        q0 = j * CH
        kv_gk = np.full(KV, -1, np.int64)
        kv_gk[0:512] = q0 + np.arange(CH)
        halo = q0 - 256 + np.arange(256)
        kv_gk[512:768] = np.where(halo >= 0, halo, -1)
        kv_gk[768:772] = np.arange(4)

        xTc = np.zeros((C, KV), np.float32)
        valid = kv_gk >= 0
        xTc[:, valid] = x[b, kv_gk[valid]].T

        cq, sq = cos_sin(q0 + np.arange(CH))
        ck, sk = cos_sin(np.maximum(kv_gk, 0))

        gq = q0 + np.arange(CH)
        mask = np.zeros((128, MTOT), np.float32)
        for c in range(7):
            rows = CBASE[c] + np.arange(CROWS[c])
            gk = kv_gk[rows]
            qw = gq[OFF_C[c]:OFF_C[c] + W_C[c]]
            real = gk >= 0
            g = np.where(real, gk, 0)[:, None]
            qq = qw[None, :]
            if c == 6:
                allow = (g <= qq) & (qq - g >= WIN)
            else:
                allow = (g <= qq) & (qq - g < WIN)
            allow &= real[:, None]
            mask[0:CROWS[c], MOFF[c]:MOFF[c] + W_C[c]] = allow
        in_maps.append({
            "xT": np.ascontiguousarray(xTc.astype(BF)),
            "wqk": wqkm, "wv": wvm, "wps": wp,
            "cos_q": cq, "sin_q": sq, "cos_k": ck, "sin_k": sk,
            "masks": np.ascontiguousarray(mask.astype(BF)),
            "p2": P2,
        })
    return in_maps


def kernel(x, w_attn, w_proj):
    from concourse import bass_utils

    x = np.asarray(x, np.float32)
    w_attn = np.asarray(w_attn, np.float32)
    w_proj = np.asarray(w_proj, np.float32)

    if "nc" not in _cache:
        _cache["nc"] = _build_nc()
    nc = _cache["nc"]

    in_maps = _host_inputs(x, w_attn, w_proj)
    res = bass_utils.run_bass_kernel_spmd(nc, in_maps, list(range(NCORES)),
                                          **_cache.get("run_kwargs", {}))
    _cache["last_result"] = res

    y = np.zeros((B, T, C), np.float32)
    for core in range(NCORES):
        b, j = core // 4, core % 4
        y[b, j * CH:(j + 1) * CH, :] = res.results[core]["outT"].T
    return y
